# revision 14
# baseline (speedup 1.0000x reference)
"""Trainium2 Bass kernel for nn_CGLayer (gnn_message_passing).

Contract: kernel(**inputs) takes FULL inputs (as reference.setup_inputs()),
returns FULL output [8,128,1,16,9] f32. Internally: data-parallel over the
batch dim across 8 NeuronCores; per core one batch element.

Algebraic reduction (exact):
  X   = conn @ vertices                  (message passing, per batch)
  Y   = mix_nl(cg(X, X))                 (per-node quadratic in X)
  S   = sum_j sph[:, j, :]               (neighbor sum commutes through the
  Z   = mix_rel(cg(Y, S))                 relative-CG stage: x-side is
  out = Z / sqrt(sum Z^2 / 16)            j-independent)

Device pipeline per core — everything node(i)-on-partition. Measured HW
facts baked in: (1) two-input DVE ops lock the shared SBUF port pair, so
GpSimd product offload fully serializes — all products stay on Vector;
(2) DMA never contends with engines, so xbar transposes overlap products
for free; (3) every DMA-completion semaphore costs ~2.1us, so the tail
(last product group + all of stage C) routes through PE transposes.

  A:  Xall[i,288]  = matmul(lhsT=connT, rhs=[vcat|vcat']), bf16, one pass;
                     products read the PSUM accumulator directly (no cast)
  S:  Ssum[i,9]    = 9 scalar-engine ACTIVATE(accum_out) ops over the
                     m-major sph layout (scalar idles during stage B)
  B:  P[i,9984]    = 13 pair products on Vector, bf16 out, packed
                     symmetry-folded slot layout; groups 0..6 -> xbar DMA
                     transpose; group 7 -> PE transposes (4 chunks per
                     PSUM tile via start=False accumulate-onto-cleared)
      Y[i,144]     = 78 bf16 matmuls lhsT=PT-chunk rhs=W2-chunk, 5 s-group
                     PSUM banks (W2 host-folds CG x w_nl, tight-packed)
  C:  per-bank P2 piece = Y_b * Ssum as soon as bank b closes (piece
      layout ordered by close time [2,3,4,0,1]); PE transposes + batched
      copyouts; 14 matmuls lhsT=P2T-chunk rhs=W3-chunk into reused PSUM
Host epilogue: unpack e=(l,c,k) columns, global per-l normalization.
"""
import numpy as np
import ml_dtypes
from math import factorial, sqrt

MAXL = 2
CH = 16
NN = 128
NB = 8
LDIM = [1, 3, 5]
FOFF = [0, 16, 64]
NF = 144
SOFF = [0, 1, 4]

# ------------------------------------------------------------- CG tables
def _cg_coeff(j1, m1, j2, m2, j3, m3):
    if m3 != m1 + m2:
        return 0.0
    pre = sqrt((2 * j3 + 1) * factorial(j3 + j1 - j2) * factorial(j3 - j1 + j2)
               * factorial(j1 + j2 - j3) / factorial(j1 + j2 + j3 + 1))
    pre *= sqrt(factorial(j3 + m3) * factorial(j3 - m3) * factorial(j1 - m1)
                * factorial(j1 + m1) * factorial(j2 - m2) * factorial(j2 + m2))
    s = 0.0
    vmin = max(0, j2 - j3 - m1, j1 - j3 + m2)
    vmax = min(j1 + j2 - j3, j1 - m1, j2 + m2)
    for v in range(vmin, vmax + 1):
        s += (-1) ** v / (factorial(v) * factorial(j1 + j2 - j3 - v)
                          * factorial(j1 - m1 - v) * factorial(j2 + m2 - v)
                          * factorial(j3 - j2 + m1 + v) * factorial(j3 - j1 - m2 + v))
    return pre * s


def _cg_matrix(l1, l2, l):
    M = np.zeros((2 * l1 + 1, 2 * l2 + 1, 2 * l + 1))
    for m1 in range(-l1, l1 + 1):
        for m2 in range(-l2, l2 + 1):
            if -l <= m1 + m2 <= l:
                M[m1 + l1, m2 + l2, m1 + m2 + l] = _cg_coeff(l1, m1, l2, m2, l, m1 + m2)
    return M


def _valid_pairs(l):
    return [(l1, l2) for l1 in range(3) for l2 in range(3)
            if abs(l1 - l2) <= l <= l1 + l2]

# ----------------------------------------------------- packed slot layout
# q = (l1, l2, m1) with l1 <= l2; for diagonal pairs m2 >= m1 (symmetric
# fold: the (m2, m1) ordering's weight folds onto the kept slot with the
# channel grid transposed). Slots of one q are contiguous over its valid,
# contiguous m2-range; each (q, m2) block is a 256-slot (c, d) grid.
def _build_qfold():
    q = []
    off = 0
    for l1 in range(3):
        for l2 in range(l1, 3):
            for m1 in range(2 * l1 + 1):
                mt1 = m1 - l1
                lo = max(0, -2 - mt1 + l2)
                hi = min(2 * l2, 2 - mt1 + l2)
                if l1 == l2:
                    lo = max(lo, m1)
                if lo > hi:
                    continue
                n = hi - lo + 1
                q.append(dict(l1=l1, l2=l2, m1=m1, m2_lo=lo, n_m2=n, off=off))
                off += 256 * n
    return q, off

Q_FOLD, NSLOT = _build_qfold()          # 13 ops, 9984 slots
NCHUNK = NSLOT // 128                   # 78
_QIDX = {(e["l1"], e["l2"], e["m1"]): e for e in Q_FOLD}
# pipeline groups, boundaries chosen to coincide with product-op bounds
# (no op splits). Last group is PE-transposed, the rest go via xbar.
GCHUNKS = [8, 10, 10, 10, 10, 8, 6, 8, 8]
GBOUND = [0]
for c in GCHUNKS:
    GBOUND.append(GBOUND[-1] + 128 * c)
assert GBOUND[-1] == NSLOT
N_XBAR_GROUPS = 4                       # groups 0..3 xbar, rest PE


def _group_ops():
    gops = [[] for _ in GCHUNKS]
    for gi in range(len(GCHUNKS)):
        a, b = GBOUND[gi], GBOUND[gi + 1]
        for qi, e in enumerate(Q_FOLD):
            s0, s1 = e["off"], e["off"] + 256 * e["n_m2"]
            lo, hi = max(a, s0), min(b, s1)
            if lo >= hi:
                continue
            j0 = (lo - s0) // 256
            j1 = (hi - s0) // 256
            gops[gi].append(dict(qi=qi, l1=e["l1"], l2=e["l2"], m1=e["m1"],
                                 m2_lo=e["m2_lo"] + j0, n_m2=j1 - j0, off=lo))
    return gops

G_OPS = _group_ops()

# Y column layout is s-group-major: col = YOFF[g] + (l - |g-2|)*16 + c'.
SG_NCOL = [16, 32, 48, 32, 16]
YOFF = [0, 16, 48, 96, 128]


def _ycol(l, m):
    g = (m - l) + 2
    return YOFF[g] + (l - abs(g - 2)) * 16


def _chunk_meta():
    meta = []
    for e in Q_FOLD:
        mt1 = e["m1"] - e["l1"]
        for j in range(e["n_m2"]):
            g = mt1 + (e["m2_lo"] + j - e["l2"]) + 2
            ncol = 16 * (3 - abs(g - 2))
            for _ in range(2):                      # 2 chunks per 256-block
                meta.append((YOFF[g], ncol, g))
    first, last = {}, {}
    for k, (_, _, g) in enumerate(meta):
        first.setdefault(g, k)
        last[g] = k
    out = []
    woff = 0
    for k, (gc0, ncol, g) in enumerate(meta):
        out.append((gc0, ncol, first[g] == k, last[g] == k, woff))
        woff += ncol
    return out, woff, last

CHUNK_META, NW2T, BANK_LAST = _chunk_meta()

# ---- stage-C P2 piece layout, ordered by bank close time
P2_ORDER = sorted(range(5), key=lambda b: BANK_LAST[b])     # close order
P2_CHUNKS = [-(-9 * SG_NCOL[b] // 128) for b in P2_ORDER]   # per-piece chunks
P2_START = [0]
for c in P2_CHUNKS:
    P2_START.append(P2_START[-1] + c)
NCH3 = P2_START[-1]                                          # 14
NP2PAD = NCH3 * 128

_CAR, _DAR = np.meshgrid(np.arange(16), np.arange(16), indexing="ij")


def _assemble_W2(w_nl):
    """W2[NSLOT, 144] f64: folded CG x w_nl; cols e = FOFF[l]+c'*LDIM[l]+k."""
    W2 = np.zeros((NSLOT, NF))
    for l in range(3):
        off_t = 0
        for (p1, p2) in _valid_pairs(l):
            Cg = _cg_matrix(p1, p2, l)
            wl = np.asarray(w_nl[l], np.float64)
            for m1 in range(2 * p1 + 1):
                for m2 in range(2 * p2 + 1):
                    st = (m1 - p1) + (m2 - p2)
                    if abs(st) > l:
                        continue
                    gc = Cg[m1, m2, st + l]
                    if gc == 0.0:
                        continue
                    if (p1 < p2) or (p1 == p2 and m1 <= m2):
                        e_ = _QIDX[(p1, p2, m1)]
                        base = e_["off"] + (m2 - e_["m2_lo"]) * 256
                        slots = base + _CAR * 16 + _DAR
                    else:
                        e_ = _QIDX[(p2, p1, m2)]
                        base = e_["off"] + (m1 - e_["m2_lo"]) * 256
                        slots = base + _DAR * 16 + _CAR
                    t = off_t + _CAR * 16 + _DAR
                    cols = YOFF[st + 2] + (l - abs(st)) * 16 + np.arange(16)
                    W2[np.ix_(slots.ravel(), cols)] += gc * wl[t.ravel(), :]
            off_t += 256
    return W2


def _assemble_W3(w_rel):
    """W3[NP2PAD, 144]: rows follow the close-ordered P2 piece layout:
    piece for bank b (in P2_ORDER) holds 9*ncol_b live rows (n-major),
    zero-padded to its chunk boundary."""
    W3full = np.zeros((9, NF, NF))       # [n, e, e']
    ar = np.arange(16)
    for l in range(3):
        off_t = 0
        for (p1, p2) in _valid_pairs(l):          # p1 = Y side, p2 = sph side
            Cg = _cg_matrix(p1, p2, l)
            wr = np.asarray(w_rel[l], np.float64)
            for m1 in range(2 * p1 + 1):
                for m2 in range(2 * p2 + 1):
                    st = (m1 - p1) + (m2 - p2)
                    if abs(st) > l:
                        continue
                    gc = Cg[m1, m2, st + l]
                    if gc == 0.0:
                        continue
                    n = SOFF[p2] + m2
                    rows = _ycol(p1, m1) + ar
                    cols = FOFF[l] + ar * LDIM[l] + (st + l)
                    W3full[np.ix_([n], rows, cols)] += gc * wr[off_t:off_t + 16, :][None]
            off_t += 16
    W3 = np.zeros((NP2PAD, NF))
    for pi, b in enumerate(P2_ORDER):
        ncol = SG_NCOL[b]
        base = P2_START[pi] * 128
        for n in range(9):
            W3[base + n * ncol: base + (n + 1) * ncol, :] = \
                W3full[n, YOFF[b]:YOFF[b] + ncol, :]
    return W3

# ------------------------------------------------------------ bass builder
_NC_CACHE = {}


def _build_nc(debug=False):
    import concourse.bacc as bacc
    import concourse.bass as bass
    import concourse.tile as tile
    from concourse import mybir
    from concourse.masks import make_identity

    f32 = mybir.dt.float32
    bf16 = mybir.dt.bfloat16
    nc = bacc.Bacc()
    d_cv = nc.declare_dram_parameter("cv", [128, 128 + 2 * NF], bf16, isOutput=False)
    d_sph = nc.declare_dram_parameter("sph", [128, 9 * 128], bf16, isOutput=False)  # [j, (m,i)]
    d_w2 = nc.declare_dram_parameter("w2", [128, NW2T], bf16, isOutput=False)
    d_w3 = nc.declare_dram_parameter("w3", [128, NCH3 * NF], bf16, isOutput=False)
    d_zout = nc.declare_dram_parameter("zout", [128, NF], f32, isOutput=True)
    if debug:
        d_dbgs = nc.declare_dram_parameter("dbgs", [128, 9], f32, isOutput=True)
        d_dbgp = nc.declare_dram_parameter("dbgp", [128, NSLOT], bf16, isOutput=True)
        d_dbgy = nc.declare_dram_parameter("dbgy", [128, NF], f32, isOutput=True)
        d_dbgp2 = nc.declare_dram_parameter("dbgp2", [128, NP2PAD], bf16, isOutput=True)

    def vap(t, doff, freedims):
        base = t[:] if not isinstance(t, bass.AP) else t
        return bass.AP(tensor=base.tensor, offset=base.offset + doff,
                       ap=[list(base.ap[0])] + [list(d) for d in freedims])

    with tile.TileContext(nc) as tc:
      with (
        tc.tile_pool(name="sb", bufs=1) as sb,
        tc.tile_pool(name="pp", bufs=8) as pp,
        tc.tile_pool(name="ptp", bufs=7) as ptp,
        tc.tile_pool(name="ps_a", bufs=1, space="PSUM") as ps_a,
        tc.tile_pool(name="ps_y", bufs=1, space="PSUM") as ps_y,
        tc.tile_pool(name="ps_t", bufs=2, space="PSUM") as ps_t,
      ):
        # ---- input DMAs. sync: cv then all xbar transposes (transpose
        # crossbar shared unit: single-dispatcher only). scalar: weights+sph.
        cv = sb.tile([128, 128 + 2 * NF], bf16)         # connT | vcat | vcat'
        nc.sync.dma_start(out=cv, in_=d_cv[:, :])
        w2 = sb.tile([128, NW2T], bf16)
        wsplit = [0, NW2T // 3, 2 * NW2T // 3, NW2T]
        for g in range(3):
            a, b = wsplit[g], wsplit[g + 1]
            nc.scalar.dma_start(out=vap(w2, a, [[1, b - a]]), in_=d_w2[:, a:b])
        sph = sb.tile([128, 9, 128], bf16)              # m-major
        nc.scalar.dma_start(
            out=sph, in_=d_sph[:, :].rearrange("p (m j) -> p m j", m=9, j=128))
        w3 = sb.tile([128, NCH3, NF], bf16)

        # ---- stage A: x_ps[i, 0:144]=X (m-inner), [i,144:288]=X' (c-inner)
        x_ps = ps_a.tile([128, 2 * NF + 16], f32)
        nc.tensor.matmul(x_ps[:, 0:2 * NF], cv[:, 0:128],
                         cv[:, 128:128 + 2 * NF], start=True, stop=True,
                         skip_group_check=True)
        # stage S: Ssum[i, m] = sum_j sphT[j, (m, i)] as 9 tiny PE matmuls
        # against a ones vector (PE idles here; start=False accumulates onto
        # the bank just cleared by the stage-A start=True)
        ones = sb.tile([128, 1], bf16)
        nc.gpsimd.memset(ones, 1.0)
        for m in range(9):
            nc.tensor.matmul(x_ps[:, 2 * NF + m: 2 * NF + m + 1],
                             sph[:, m, :], ones, start=False, stop=(m == 8),
                             skip_group_check=True)

        # X' (c-inner half) to SBUF bf16: TensorTensor may read only one
        # input from PSUM, so in1 comes from SBUF while in0 stays in PSUM
        Xc = sb.tile([128, NF], bf16)
        nc.vector.tensor_copy(out=Xc, in_=x_ps[:, NF:2 * NF])

        ssum = sb.tile([128, 9], f32)
        # ---- ERep[i, qi, (c,16d)] = X[i, FOFF[l1]+m1+c*LDIM[l1]] x16:
        # materialized in0 gives every product op unit innermost strides,
        # which engages the DVE 2x bf16 mode (measured 0.55x per op).
        # Scalar builds it from the PSUM X, chasing ahead of the products.
        erep = sb.tile([128, 13, 256], bf16)
        for qi, e in enumerate(Q_FOLD):
            nc.scalar.copy(
                out=erep[:, qi, :],
                in_=vap(x_ps, FOFF[e["l1"]] + e["m1"],
                        [[LDIM[e["l1"]], 16], [0, 16]]))
        # w3 dispatched after the ERep chase; needed only by stage C
        nc.scalar.dma_start(
            out=w3, in_=d_w3[:, :].rearrange("p (c e) -> p c e", c=NCH3, e=NF))
        nc.scalar.copy(out=ssum, in_=x_ps[:, 2 * NF:2 * NF + 9])
        ident = sb.tile([128, 128], bf16)
        make_identity(nc, ident)
        # P2 laid out piece-major in close order; pad cols zeroed up front
        P2 = sb.tile([128, NP2PAD], bf16)
        for pi, b in enumerate(P2_ORDER):
            live = 9 * SG_NCOL[b]
            lo, hi = P2_START[pi] * 128 + live, P2_START[pi + 1] * 128
            if hi > lo:
                nc.gpsimd.memset(P2[:, lo:hi], 0.0)
        P2T = sb.tile([128, NCH3, 128], bf16)

        def products(gi):
            gbase = GBOUND[gi]
            gslots = GBOUND[gi + 1] - gbase
            P = pp.tile([128, 2048], bf16)
            for op in G_OPS[gi]:
                l1, l2, m1 = op["l1"], op["l2"], op["m1"]
                nm2 = op["n_m2"]
                nc.vector.tensor_tensor(
                    out=vap(P, op["off"] - gbase,
                            [[256, nm2], [16, 16], [1, 16]]),
                    in0=vap(erep, op["qi"] * 256,
                            [[0, nm2], [16, 16], [1, 16]]),
                    in1=vap(Xc, FOFF[l2] + op["m2_lo"] * 16,
                            [[16, nm2], [0, 16], [1, 16]]),
                    op=mybir.AluOpType.mult)
            return P, gslots

        def ymm(k, PTap):
            gc0, ncol, st_f, sp_f, woff = CHUNK_META[k]
            nc.tensor.matmul(ymixg[YOFF.index(gc0)], PTap,
                             w2[:, woff:woff + ncol], start=st_f, stop=sp_f)

        # PE-transpose a run of chunks of `src` into `dst[:, k0f:...]`,
        # packing `per` chunks per PSUM tile (start=False accumulates onto
        # the bank cleared by the tile's first start=True transpose).
        cp_rr = [0]
        def pe_transpose(src, soff, nch, dst, dchunk0, cp="alt"):
            done = 0
            while done < nch:
                per = min(4, nch - done)
                t_ps = ps_t.tile([128, 512], bf16)
                for j in range(per):
                    nc.tensor.matmul(
                        t_ps[:, j * 128:(j + 1) * 128],
                        src[:, soff + (done + j) * 128: soff + (done + j + 1) * 128],
                        ident, is_transpose=True,
                        start=(j == 0), stop=(j == per - 1),
                        skip_group_check=True)
                c0 = dchunk0 + done
                dstap = vap(dst, c0 * 128, [[1, per * 128]])
                use_s = cp == "s" or (cp == "alt" and cp_rr[0] % 2 == 0)
                if use_s:
                    nc.scalar.copy(out=dstap, in_=t_ps[:, 0:per * 128])
                else:
                    nc.vector.tensor_copy(out=dstap, in_=t_ps[:, 0:per * 128])
                cp_rr[0] += 1
                done += per

        # ---- stage B
        ymixg = [ps_y.tile([128, SG_NCOL[g]], f32, name=f"ymix{g}")
                 for g in range(5)]
        for gi in range(N_XBAR_GROUPS):
            P, gslots = products(gi)
            nch = gslots // 128
            PT = ptp.tile([128, 16, 128], bf16)
            nc.sync.dma_start(out=PT[:, 0:nch, :], in_=P[:, 0:gslots],
                              transpose=True)
            if debug:
                nc.scalar.dma_start(
                    out=d_dbgp[:, GBOUND[gi]:GBOUND[gi] + gslots],
                    in_=P[:, 0:gslots])
            for c in range(nch):
                ymm(GBOUND[gi] // 128 + c, PT[:, c, :])
        # tail groups via PE transposes (no DMA-completion latency on tail)
        n_pe_chunks = (NSLOT - GBOUND[N_XBAR_GROUPS]) // 128
        PT7 = sb.tile([128, n_pe_chunks, 128], bf16)
        pt7c = 0
        for gi in range(N_XBAR_GROUPS, len(GCHUNKS)):
            P, gslots = products(gi)
            nch = gslots // 128
            last_pe = gi >= N_XBAR_GROUPS + 2
            pe_transpose(P, 0, nch, PT7, pt7c, cp=("v" if last_pe else "s"))
            if debug:
                nc.scalar.dma_start(
                    out=d_dbgp[:, GBOUND[gi]:GBOUND[gi] + gslots],
                    in_=P[:, 0:gslots])
            for c in range(nch):
                ymm(GBOUND[gi] // 128 + c, PT7[:, pt7c + c, :])
            pt7c += nch

        # ---- stage C: per-piece P2 = Y_b * Ssum, PE transposes, 14 matmuls
        for pi, b in enumerate(P2_ORDER):
            ncol = SG_NCOL[b]
            nc.vector.tensor_tensor(
                out=vap(P2, P2_START[pi] * 128, [[ncol, 9], [1, ncol]]),
                in0=vap(ymixg[b], 0, [[0, 9], [1, ncol]]),
                in1=vap(ssum, 0, [[1, 9], [0, ncol]]),
                op=mybir.AluOpType.mult)
            pe_transpose(P2, P2_START[pi] * 128, P2_CHUNKS[pi], P2T,
                         P2_START[pi], cp="alt")
        z_ps = x_ps                     # bank reuse: X consumed by products
        for c in range(NCH3):
            nc.tensor.matmul(z_ps[:, 0:NF], P2T[:, c, :], w3[:, c, :],
                             start=(c == 0), stop=(c == NCH3 - 1))
        zsb = sb.tile([128, NF], f32)
        nc.scalar.activation(zsb, z_ps[:, 0:NF],
                             mybir.ActivationFunctionType.Copy)
        nc.sync.dma_start(out=d_zout[:, :], in_=zsb)

        if debug:
            nc.sync.dma_start(out=d_dbgs[:, :], in_=ssum)
            ydbg = sb.tile([128, NF], f32)
            for g in range(5):
                nc.vector.tensor_copy(
                    out=ydbg[:, YOFF[g]:YOFF[g] + SG_NCOL[g]], in_=ymixg[g])
            nc.sync.dma_start(out=d_dbgy[:, :], in_=ydbg)
            nc.sync.dma_start(out=d_dbgp2[:, :], in_=P2)

    nc.compile()
    return nc

# ------------------------------------------------------------- host entry
LAST_RESULT = {}


def _get_nc():
    if "nc" not in _NC_CACHE:
        _NC_CACHE["nc"] = _build_nc()
    return _NC_CACHE["nc"]


def _pack_w2_tight(W2):
    """[NSLOT, 144] -> [128, NW2T] bf16: per chunk only its live columns."""
    out = np.zeros((128, NW2T))
    for k, (gc0, ncol, _, _, woff) in enumerate(CHUNK_META):
        out[:, woff:woff + ncol] = W2[k * 128:(k + 1) * 128, gc0:gc0 + ncol]
    return out.astype(ml_dtypes.bfloat16)


def _pack_chunked(W, nchunk):
    """[nchunk*128, e] -> [128, nchunk*e] bf16 (chunk-major per partition)."""
    e = W.shape[1]
    return np.ascontiguousarray(
        W.reshape(nchunk, 128, e).transpose(1, 0, 2)
        .astype(ml_dtypes.bfloat16).reshape(128, nchunk * e))


def kernel(vertices_0, vertices_1, vertices_2, connectivity,
           sph_0, sph_1, sph_2,
           w_nl_0, w_nl_1, w_nl_2,
           w_rel_0, w_rel_1, w_rel_2):
    from concourse.bass_utils import run_bass_kernel_spmd

    f = np.float32
    verts = [np.asarray(v, f) for v in (vertices_0, vertices_1, vertices_2)]
    sphs = [np.asarray(s, f) for s in (sph_0, sph_1, sph_2)]
    conn = np.asarray(connectivity)
    W2 = _assemble_W2([np.asarray(w, f) for w in (w_nl_0, w_nl_1, w_nl_2)])
    W3 = _assemble_W3([np.asarray(w, f) for w in (w_rel_0, w_rel_1, w_rel_2)])
    w2p = _pack_w2_tight(W2)
    w3p = _pack_chunked(W3, NCH3)

    in_maps = []
    for b in range(NB):
        vcat = np.concatenate([v[b].reshape(128, -1) for v in verts], axis=1)
        vcat_t = np.concatenate(
            [v[b].reshape(128, CH, LDIM[l]).transpose(0, 2, 1).reshape(128, -1)
             for l, v in enumerate(verts)], axis=1)
        cv = np.concatenate([conn[b].astype(f).T, vcat, vcat_t], axis=1)
        sph_cat = np.concatenate([s[b][:, :, 0, :] for s in sphs], axis=-1)
        sph_mj = sph_cat.transpose(1, 2, 0).reshape(128, 9 * 128)  # [j,(m,i)]
        in_maps.append(dict(
            cv=np.ascontiguousarray(cv.astype(ml_dtypes.bfloat16)),
            sph=np.ascontiguousarray(sph_mj.astype(ml_dtypes.bfloat16)),
            w2=w2p, w3=w3p))

    res = run_bass_kernel_spmd(_get_nc(), in_maps, list(range(NB)))
    LAST_RESULT["res"] = res
    Z = np.stack([res.results[b]["zout"] for b in range(NB)])   # [8, 128, 144]

    # host epilogue: unpack e=(l,c,k) cols, global per-l normalization
    out = np.zeros((NB, 128, 1, 16, 9), dtype=f)
    koff = [0, 1, 4]
    for l in range(3):
        cols = FOFF[l] + (np.arange(16)[:, None] * LDIM[l]
                          + np.arange(LDIM[l])[None, :])
        blk = Z[:, :, cols]                                     # [8,128,16,ld]
        nf = np.sum(blk.astype(np.float64) ** 2)
        out[:, :, 0, :, koff[l]:koff[l] + LDIM[l]] = blk / np.sqrt(nf / 16.0)
    return out


# revision 17
# speedup vs baseline: 1.0250x; 1.0250x over previous
"""Trainium2 Bass kernel for nn_CGLayer (gnn_message_passing).

Contract: kernel(**inputs) takes FULL inputs (as reference.setup_inputs()),
returns FULL output [8,128,1,16,9] f32. Internally: data-parallel over the
batch dim across 8 NeuronCores; per core one batch element.

Algebraic reduction (exact):
  X   = conn @ vertices                  (message passing, per batch)
  Y   = mix_nl(cg(X, X))                 (per-node quadratic in X)
  S   = sum_j sph[:, j, :]               (neighbor sum commutes through the
  Z   = mix_rel(cg(Y, S))                 relative-CG stage: x-side is
  out = Z / sqrt(sum Z^2 / 16)            j-independent)

Device pipeline per core — everything node(i)-on-partition. Measured HW
facts baked in: (1) two-input DVE ops lock the shared SBUF port pair, so
GpSimd product offload fully serializes — all products stay on Vector;
(2) DMA never contends with engines, so xbar transposes overlap products
for free; (3) every DMA-completion semaphore costs ~2.1us, so the tail
(last product group + all of stage C) routes through PE transposes.

  A:  Xall[i,288]  = matmul(lhsT=connT, rhs=[vcat|vcat']), bf16, one pass;
                     products read the PSUM accumulator directly (no cast)
  S:  Ssum[i,9]    = 9 scalar-engine ACTIVATE(accum_out) ops over the
                     m-major sph layout (scalar idles during stage B)
  B:  P[i,9984]    = 13 pair products on Vector, bf16 out, packed
                     symmetry-folded slot layout; groups 0..6 -> xbar DMA
                     transpose; group 7 -> PE transposes (4 chunks per
                     PSUM tile via start=False accumulate-onto-cleared)
      Y[i,144]     = 78 bf16 matmuls lhsT=PT-chunk rhs=W2-chunk, 5 s-group
                     PSUM banks (W2 host-folds CG x w_nl, tight-packed)
  C:  per-bank P2 piece = Y_b * Ssum as soon as bank b closes (piece
      layout ordered by close time [2,3,4,0,1]); PE transposes + batched
      copyouts; 14 matmuls lhsT=P2T-chunk rhs=W3-chunk into reused PSUM
Host epilogue: unpack e=(l,c,k) columns, global per-l normalization.
"""
import numpy as np
import ml_dtypes
from math import factorial, sqrt

MAXL = 2
CH = 16
NN = 128
NB = 8
LDIM = [1, 3, 5]
FOFF = [0, 16, 64]
NF = 144
SOFF = [0, 1, 4]

# ------------------------------------------------------------- CG tables
def _cg_coeff(j1, m1, j2, m2, j3, m3):
    if m3 != m1 + m2:
        return 0.0
    pre = sqrt((2 * j3 + 1) * factorial(j3 + j1 - j2) * factorial(j3 - j1 + j2)
               * factorial(j1 + j2 - j3) / factorial(j1 + j2 + j3 + 1))
    pre *= sqrt(factorial(j3 + m3) * factorial(j3 - m3) * factorial(j1 - m1)
                * factorial(j1 + m1) * factorial(j2 - m2) * factorial(j2 + m2))
    s = 0.0
    vmin = max(0, j2 - j3 - m1, j1 - j3 + m2)
    vmax = min(j1 + j2 - j3, j1 - m1, j2 + m2)
    for v in range(vmin, vmax + 1):
        s += (-1) ** v / (factorial(v) * factorial(j1 + j2 - j3 - v)
                          * factorial(j1 - m1 - v) * factorial(j2 + m2 - v)
                          * factorial(j3 - j2 + m1 + v) * factorial(j3 - j1 - m2 + v))
    return pre * s


def _cg_matrix(l1, l2, l):
    M = np.zeros((2 * l1 + 1, 2 * l2 + 1, 2 * l + 1))
    for m1 in range(-l1, l1 + 1):
        for m2 in range(-l2, l2 + 1):
            if -l <= m1 + m2 <= l:
                M[m1 + l1, m2 + l2, m1 + m2 + l] = _cg_coeff(l1, m1, l2, m2, l, m1 + m2)
    return M


def _valid_pairs(l):
    return [(l1, l2) for l1 in range(3) for l2 in range(3)
            if abs(l1 - l2) <= l <= l1 + l2]

# ----------------------------------------------------- packed slot layout
# q = (l1, l2, m1) with l1 <= l2; for diagonal pairs m2 >= m1 (symmetric
# fold: the (m2, m1) ordering's weight folds onto the kept slot with the
# channel grid transposed). Slots of one q are contiguous over its valid,
# contiguous m2-range; each (q, m2) block is a 256-slot (c, d) grid.
def _build_qfold():
    q = []
    off = 0
    for l1 in range(3):
        for l2 in range(l1, 3):
            for m1 in range(2 * l1 + 1):
                mt1 = m1 - l1
                lo = max(0, -2 - mt1 + l2)
                hi = min(2 * l2, 2 - mt1 + l2)
                if l1 == l2:
                    lo = max(lo, m1)
                if lo > hi:
                    continue
                n = hi - lo + 1
                q.append(dict(l1=l1, l2=l2, m1=m1, m2_lo=lo, n_m2=n, off=off))
                off += 256 * n
    return q, off

Q_FOLD, NSLOT = _build_qfold()          # 13 ops, 9984 slots
NCHUNK = NSLOT // 128                   # 78
_QIDX = {(e["l1"], e["l2"], e["m1"]): e for e in Q_FOLD}
# pipeline groups, boundaries chosen to coincide with product-op bounds
# (no op splits). Last group is PE-transposed, the rest go via xbar.
GCHUNKS = [8, 10, 10, 10, 10, 8, 6, 8, 8]
GBOUND = [0]
for c in GCHUNKS:
    GBOUND.append(GBOUND[-1] + 128 * c)
assert GBOUND[-1] == NSLOT
N_XBAR_GROUPS = 4                       # groups 0..3 xbar, rest PE


def _group_ops():
    gops = [[] for _ in GCHUNKS]
    for gi in range(len(GCHUNKS)):
        a, b = GBOUND[gi], GBOUND[gi + 1]
        for qi, e in enumerate(Q_FOLD):
            s0, s1 = e["off"], e["off"] + 256 * e["n_m2"]
            lo, hi = max(a, s0), min(b, s1)
            if lo >= hi:
                continue
            j0 = (lo - s0) // 256
            j1 = (hi - s0) // 256
            gops[gi].append(dict(qi=qi, l1=e["l1"], l2=e["l2"], m1=e["m1"],
                                 m2_lo=e["m2_lo"] + j0, n_m2=j1 - j0, off=lo))
    return gops

G_OPS = _group_ops()

# Y column layout is s-group-major: col = YOFF[g] + (l - |g-2|)*16 + c'.
SG_NCOL = [16, 32, 48, 32, 16]
YOFF = [0, 16, 48, 96, 128]


def _ycol(l, m):
    g = (m - l) + 2
    return YOFF[g] + (l - abs(g - 2)) * 16


def _chunk_meta():
    meta = []
    for e in Q_FOLD:
        mt1 = e["m1"] - e["l1"]
        for j in range(e["n_m2"]):
            g = mt1 + (e["m2_lo"] + j - e["l2"]) + 2
            ncol = 16 * (3 - abs(g - 2))
            for _ in range(2):                      # 2 chunks per 256-block
                meta.append((YOFF[g], ncol, g))
    first, last = {}, {}
    for k, (_, _, g) in enumerate(meta):
        first.setdefault(g, k)
        last[g] = k
    out = []
    woff = 0
    for k, (gc0, ncol, g) in enumerate(meta):
        out.append((gc0, ncol, first[g] == k, last[g] == k, woff))
        woff += ncol
    return out, woff, last

CHUNK_META, NW2T, BANK_LAST = _chunk_meta()

# ---- stage-C P2 piece layout, ordered by bank close time
P2_ORDER = sorted(range(5), key=lambda b: BANK_LAST[b])     # close order
P2_CHUNKS = [-(-9 * SG_NCOL[b] // 128) for b in P2_ORDER]   # per-piece chunks
P2_START = [0]
for c in P2_CHUNKS:
    P2_START.append(P2_START[-1] + c)
NCH3 = P2_START[-1]                                          # 14
NP2PAD = NCH3 * 128

_CAR, _DAR = np.meshgrid(np.arange(16), np.arange(16), indexing="ij")


def _assemble_W2(w_nl):
    """W2[NSLOT, 144] f64: folded CG x w_nl; cols e = FOFF[l]+c'*LDIM[l]+k."""
    W2 = np.zeros((NSLOT, NF))
    for l in range(3):
        off_t = 0
        for (p1, p2) in _valid_pairs(l):
            Cg = _cg_matrix(p1, p2, l)
            wl = np.asarray(w_nl[l], np.float64)
            for m1 in range(2 * p1 + 1):
                for m2 in range(2 * p2 + 1):
                    st = (m1 - p1) + (m2 - p2)
                    if abs(st) > l:
                        continue
                    gc = Cg[m1, m2, st + l]
                    if gc == 0.0:
                        continue
                    if (p1 < p2) or (p1 == p2 and m1 <= m2):
                        e_ = _QIDX[(p1, p2, m1)]
                        base = e_["off"] + (m2 - e_["m2_lo"]) * 256
                        slots = base + _CAR * 16 + _DAR
                    else:
                        e_ = _QIDX[(p2, p1, m2)]
                        base = e_["off"] + (m1 - e_["m2_lo"]) * 256
                        slots = base + _DAR * 16 + _CAR
                    t = off_t + _CAR * 16 + _DAR
                    cols = YOFF[st + 2] + (l - abs(st)) * 16 + np.arange(16)
                    W2[np.ix_(slots.ravel(), cols)] += gc * wl[t.ravel(), :]
            off_t += 256
    return W2


def _assemble_W3(w_rel):
    """W3[NP2PAD, 144]: rows follow the close-ordered P2 piece layout:
    piece for bank b (in P2_ORDER) holds 9*ncol_b live rows (n-major),
    zero-padded to its chunk boundary."""
    W3full = np.zeros((9, NF, NF))       # [n, e, e']
    ar = np.arange(16)
    for l in range(3):
        off_t = 0
        for (p1, p2) in _valid_pairs(l):          # p1 = Y side, p2 = sph side
            Cg = _cg_matrix(p1, p2, l)
            wr = np.asarray(w_rel[l], np.float64)
            for m1 in range(2 * p1 + 1):
                for m2 in range(2 * p2 + 1):
                    st = (m1 - p1) + (m2 - p2)
                    if abs(st) > l:
                        continue
                    gc = Cg[m1, m2, st + l]
                    if gc == 0.0:
                        continue
                    n = SOFF[p2] + m2
                    rows = _ycol(p1, m1) + ar
                    cols = FOFF[l] + ar * LDIM[l] + (st + l)
                    W3full[np.ix_([n], rows, cols)] += gc * wr[off_t:off_t + 16, :][None]
            off_t += 16
    W3 = np.zeros((NP2PAD, NF))
    for pi, b in enumerate(P2_ORDER):
        ncol = SG_NCOL[b]
        base = P2_START[pi] * 128
        for n in range(9):
            W3[base + n * ncol: base + (n + 1) * ncol, :] = \
                W3full[n, YOFF[b]:YOFF[b] + ncol, :]
    return W3

# ------------------------------------------------------------ bass builder
_NC_CACHE = {}


def _build_nc(debug=False):
    import concourse.bacc as bacc
    import concourse.bass as bass
    import concourse.tile as tile
    from concourse import mybir
    from concourse.masks import make_identity

    f32 = mybir.dt.float32
    bf16 = mybir.dt.bfloat16
    nc = bacc.Bacc()
    d_cv = nc.declare_dram_parameter("cv", [128, 128 + 2 * NF], bf16, isOutput=False)
    d_sph = nc.declare_dram_parameter("sph", [128, 9 * 128], bf16, isOutput=False)  # [j, (m,i)]
    d_w2 = nc.declare_dram_parameter("w2", [128, NW2T], bf16, isOutput=False)
    d_w3 = nc.declare_dram_parameter("w3", [128, NCH3 * NF], bf16, isOutput=False)
    d_zout = nc.declare_dram_parameter("zout", [128, NF], f32, isOutput=True)
    if debug:
        d_dbgs = nc.declare_dram_parameter("dbgs", [128, 9], f32, isOutput=True)
        d_dbgp = nc.declare_dram_parameter("dbgp", [128, NSLOT], bf16, isOutput=True)
        d_dbgy = nc.declare_dram_parameter("dbgy", [128, NF], f32, isOutput=True)
        d_dbgp2 = nc.declare_dram_parameter("dbgp2", [128, NP2PAD], bf16, isOutput=True)

    def vap(t, doff, freedims):
        base = t[:] if not isinstance(t, bass.AP) else t
        return bass.AP(tensor=base.tensor, offset=base.offset + doff,
                       ap=[list(base.ap[0])] + [list(d) for d in freedims])

    with tile.TileContext(nc) as tc:
      with (
        tc.tile_pool(name="sb", bufs=1) as sb,
        tc.tile_pool(name="pp", bufs=8) as pp,
        tc.tile_pool(name="ptp", bufs=7) as ptp,
        tc.tile_pool(name="ps_a", bufs=1, space="PSUM") as ps_a,
        tc.tile_pool(name="ps_y", bufs=1, space="PSUM") as ps_y,
        tc.tile_pool(name="ps_t", bufs=2, space="PSUM") as ps_t,
      ):
        # ---- input DMAs. sync: cv then all xbar transposes (transpose
        # crossbar shared unit: single-dispatcher only). scalar: weights+sph.
        cv = sb.tile([128, 128 + 2 * NF], bf16)         # connT | vcat | vcat'
        nc.sync.dma_start(out=cv, in_=d_cv[:, :])
        sph = sb.tile([128, 9, 128], bf16)              # [i, m, j]
        nc.scalar.dma_start(
            out=sph, in_=d_sph[:, :].rearrange("p (m j) -> p m j", m=9, j=128))
        w2 = sb.tile([128, NW2T], bf16)
        wsplit = [0, NW2T // 3, 2 * NW2T // 3, NW2T]
        for g in range(3):
            a, b = wsplit[g], wsplit[g + 1]
            nc.scalar.dma_start(out=vap(w2, a, [[1, b - a]]), in_=d_w2[:, a:b])
        w3 = sb.tile([128, NCH3, NF], bf16)

        # ---- stage A: x_ps[i, 0:144]=X (m-inner), [i,144:288]=X' (c-inner)
        x_ps = ps_a.tile([128, 2 * NF], f32)
        nc.tensor.matmul(x_ps, cv[:, 0:128], cv[:, 128:128 + 2 * NF],
                         start=True, stop=True)
        # stage S: Ssum[i, 9] = one DVE free-axis reduce, before products
        # (sph is dispatched first so its DMA lands by ~10.6)
        ssum = sb.tile([128, 9], f32)
        nc.vector.tensor_reduce(ssum, sph[:, :, :], mybir.AxisListType.X,
                                mybir.AluOpType.add)

        # X' (c-inner half) to SBUF bf16: TensorTensor may read only one
        # input from PSUM, so in1 comes from SBUF while in0 stays in PSUM
        Xc = sb.tile([128, NF], bf16)
        nc.vector.tensor_copy(out=Xc, in_=x_ps[:, NF:2 * NF])

        # ---- ERep[i, qi, (c,16d)] = X[i, FOFF[l1]+m1+c*LDIM[l1]] x16:
        # materialized in0 gives every product op unit innermost strides,
        # which engages the DVE 2x bf16 mode (measured 0.55x per op).
        # Scalar builds it from the PSUM X, chasing ahead of the products.
        erep = sb.tile([128, 13, 256], bf16)
        for qi, e in enumerate(Q_FOLD):
            nc.scalar.copy(
                out=erep[:, qi, :],
                in_=vap(x_ps, FOFF[e["l1"]] + e["m1"],
                        [[LDIM[e["l1"]], 16], [0, 16]]))
        # w3 dispatched after the ERep chase; needed only by stage C
        nc.scalar.dma_start(
            out=w3, in_=d_w3[:, :].rearrange("p (c e) -> p c e", c=NCH3, e=NF))
        ident = sb.tile([128, 128], bf16)
        make_identity(nc, ident)
        # P2 laid out piece-major in close order; pad cols zeroed up front
        P2 = sb.tile([128, NP2PAD], bf16)
        for pi, b in enumerate(P2_ORDER):
            live = 9 * SG_NCOL[b]
            lo, hi = P2_START[pi] * 128 + live, P2_START[pi + 1] * 128
            if hi > lo:
                nc.gpsimd.memset(P2[:, lo:hi], 0.0)
        P2T = sb.tile([128, NCH3, 128], bf16)

        def products(gi):
            gbase = GBOUND[gi]
            gslots = GBOUND[gi + 1] - gbase
            P = pp.tile([128, 2048], bf16)
            for op in G_OPS[gi]:
                l1, l2, m1 = op["l1"], op["l2"], op["m1"]
                nm2 = op["n_m2"]
                nc.vector.tensor_tensor(
                    out=vap(P, op["off"] - gbase,
                            [[256, nm2], [16, 16], [1, 16]]),
                    in0=vap(erep, op["qi"] * 256,
                            [[0, nm2], [16, 16], [1, 16]]),
                    in1=vap(Xc, FOFF[l2] + op["m2_lo"] * 16,
                            [[16, nm2], [0, 16], [1, 16]]),
                    op=mybir.AluOpType.mult)
            return P, gslots

        def ymm(k, PTap):
            gc0, ncol, st_f, sp_f, woff = CHUNK_META[k]
            nc.tensor.matmul(ymixg[YOFF.index(gc0)], PTap,
                             w2[:, woff:woff + ncol], start=st_f, stop=sp_f)

        # PE-transpose a run of chunks of `src` into `dst[:, k0f:...]`,
        # packing `per` chunks per PSUM tile (start=False accumulates onto
        # the bank cleared by the tile's first start=True transpose).
        cp_rr = [0]
        def pe_transpose(src, soff, nch, dst, dchunk0, cp="alt"):
            done = 0
            while done < nch:
                per = min(4, nch - done)
                t_ps = ps_t.tile([128, 512], bf16)
                for j in range(per):
                    nc.tensor.matmul(
                        t_ps[:, j * 128:(j + 1) * 128],
                        src[:, soff + (done + j) * 128: soff + (done + j + 1) * 128],
                        ident, is_transpose=True,
                        start=(j == 0), stop=(j == per - 1),
                        skip_group_check=True)
                c0 = dchunk0 + done
                dstap = vap(dst, c0 * 128, [[1, per * 128]])
                use_s = cp == "s" or (cp == "alt" and cp_rr[0] % 2 == 0)
                if use_s:
                    nc.scalar.copy(out=dstap, in_=t_ps[:, 0:per * 128])
                else:
                    nc.vector.tensor_copy(out=dstap, in_=t_ps[:, 0:per * 128])
                cp_rr[0] += 1
                done += per

        # ---- stage B
        ymixg = [ps_y.tile([128, SG_NCOL[g]], f32, name=f"ymix{g}")
                 for g in range(5)]
        xpt = []
        for gi in range(N_XBAR_GROUPS):
            P, gslots = products(gi)
            nch = gslots // 128
            PT = ptp.tile([128, 16, 128], bf16)
            nc.sync.dma_start(out=PT[:, 0:nch, :], in_=P[:, 0:gslots],
                              transpose=True)
            xpt.append(PT)
            if debug:
                nc.scalar.dma_start(
                    out=d_dbgp[:, GBOUND[gi]:GBOUND[gi] + gslots],
                    in_=P[:, 0:gslots])
        # tail groups via PE transposes (no DMA-completion latency on tail).
        # All ymix matmuls are emitted AFTER every transpose: the PE queue
        # executes in order, and sem-gated xbar matmuls emitted early would
        # head-of-line-block the PE transposes.
        n_pe_chunks = (NSLOT - GBOUND[N_XBAR_GROUPS]) // 128
        PT7 = sb.tile([128, n_pe_chunks, 128], bf16)
        pt7c = 0
        for gi in range(N_XBAR_GROUPS, len(GCHUNKS)):
            P, gslots = products(gi)
            nch = gslots // 128
            last_pe = gi >= N_XBAR_GROUPS + 2
            pe_transpose(P, 0, nch, PT7, pt7c, cp=("v" if last_pe else "s"))
            if debug:
                nc.scalar.dma_start(
                    out=d_dbgp[:, GBOUND[gi]:GBOUND[gi] + gslots],
                    in_=P[:, 0:gslots])
            pt7c += nch
        for gi in range(len(GCHUNKS)):
            nch = (GBOUND[gi + 1] - GBOUND[gi]) // 128
            for c in range(nch):
                k = GBOUND[gi] // 128 + c
                if gi < N_XBAR_GROUPS:
                    ymm(k, xpt[gi][:, c, :])
                else:
                    ymm(k, PT7[:, k - GBOUND[N_XBAR_GROUPS] // 128, :])

        # ---- stage C: per-piece P2 = Y_b * Ssum, PE transposes, 14 matmuls
        for pi, b in enumerate(P2_ORDER):
            ncol = SG_NCOL[b]
            nc.vector.tensor_tensor(
                out=vap(P2, P2_START[pi] * 128, [[ncol, 9], [1, ncol]]),
                in0=vap(ymixg[b], 0, [[0, 9], [1, ncol]]),
                in1=vap(ssum, 0, [[1, 9], [0, ncol]]),
                op=mybir.AluOpType.mult)
            pe_transpose(P2, P2_START[pi] * 128, P2_CHUNKS[pi], P2T,
                         P2_START[pi], cp="alt")
        z_ps = x_ps                     # bank reuse: X consumed by products
        for c in range(NCH3):
            nc.tensor.matmul(z_ps[:, 0:NF], P2T[:, c, :], w3[:, c, :],
                             start=(c == 0), stop=(c == NCH3 - 1))
        zsb = sb.tile([128, NF], f32)
        nc.scalar.activation(zsb, z_ps[:, 0:NF],
                             mybir.ActivationFunctionType.Copy)
        nc.sync.dma_start(out=d_zout[:, :], in_=zsb)

        if debug:
            nc.sync.dma_start(out=d_dbgs[:, :], in_=ssum)
            ydbg = sb.tile([128, NF], f32)
            for g in range(5):
                nc.vector.tensor_copy(
                    out=ydbg[:, YOFF[g]:YOFF[g] + SG_NCOL[g]], in_=ymixg[g])
            nc.sync.dma_start(out=d_dbgy[:, :], in_=ydbg)
            nc.sync.dma_start(out=d_dbgp2[:, :], in_=P2)

    nc.compile()
    return nc

# ------------------------------------------------------------- host entry
LAST_RESULT = {}


def _get_nc():
    if "nc" not in _NC_CACHE:
        _NC_CACHE["nc"] = _build_nc()
    return _NC_CACHE["nc"]


def _pack_w2_tight(W2):
    """[NSLOT, 144] -> [128, NW2T] bf16: per chunk only its live columns."""
    out = np.zeros((128, NW2T))
    for k, (gc0, ncol, _, _, woff) in enumerate(CHUNK_META):
        out[:, woff:woff + ncol] = W2[k * 128:(k + 1) * 128, gc0:gc0 + ncol]
    return out.astype(ml_dtypes.bfloat16)


def _pack_chunked(W, nchunk):
    """[nchunk*128, e] -> [128, nchunk*e] bf16 (chunk-major per partition)."""
    e = W.shape[1]
    return np.ascontiguousarray(
        W.reshape(nchunk, 128, e).transpose(1, 0, 2)
        .astype(ml_dtypes.bfloat16).reshape(128, nchunk * e))


def kernel(vertices_0, vertices_1, vertices_2, connectivity,
           sph_0, sph_1, sph_2,
           w_nl_0, w_nl_1, w_nl_2,
           w_rel_0, w_rel_1, w_rel_2):
    from concourse.bass_utils import run_bass_kernel_spmd

    f = np.float32
    verts = [np.asarray(v, f) for v in (vertices_0, vertices_1, vertices_2)]
    sphs = [np.asarray(s, f) for s in (sph_0, sph_1, sph_2)]
    conn = np.asarray(connectivity)
    W2 = _assemble_W2([np.asarray(w, f) for w in (w_nl_0, w_nl_1, w_nl_2)])
    W3 = _assemble_W3([np.asarray(w, f) for w in (w_rel_0, w_rel_1, w_rel_2)])
    w2p = _pack_w2_tight(W2)
    w3p = _pack_chunked(W3, NCH3)

    in_maps = []
    for b in range(NB):
        vcat = np.concatenate([v[b].reshape(128, -1) for v in verts], axis=1)
        vcat_t = np.concatenate(
            [v[b].reshape(128, CH, LDIM[l]).transpose(0, 2, 1).reshape(128, -1)
             for l, v in enumerate(verts)], axis=1)
        cv = np.concatenate([conn[b].astype(f).T, vcat, vcat_t], axis=1)
        sph_cat = np.concatenate([s[b][:, :, 0, :] for s in sphs], axis=-1)
        sph_mj = sph_cat.transpose(0, 2, 1).reshape(128, 9 * 128)  # [i,(m,j)]
        in_maps.append(dict(
            cv=np.ascontiguousarray(cv.astype(ml_dtypes.bfloat16)),
            sph=np.ascontiguousarray(sph_mj.astype(ml_dtypes.bfloat16)),
            w2=w2p, w3=w3p))

    res = run_bass_kernel_spmd(_get_nc(), in_maps, list(range(NB)))
    LAST_RESULT["res"] = res
    Z = np.stack([res.results[b]["zout"] for b in range(NB)])   # [8, 128, 144]

    # host epilogue: unpack e=(l,c,k) cols, global per-l normalization
    out = np.zeros((NB, 128, 1, 16, 9), dtype=f)
    koff = [0, 1, 4]
    for l in range(3):
        cols = FOFF[l] + (np.arange(16)[:, None] * LDIM[l]
                          + np.arange(LDIM[l])[None, :])
        blk = Z[:, :, cols]                                     # [8,128,16,ld]
        nf = np.sum(blk.astype(np.float64) ** 2)
        out[:, :, 0, :, koff[l]:koff[l] + LDIM[l]] = blk / np.sqrt(nf / 16.0)
    return out


# revision 18
# speedup vs baseline: 1.0280x; 1.0029x over previous
"""Trainium2 Bass kernel for nn_CGLayer (gnn_message_passing).

Contract: kernel(**inputs) takes FULL inputs (as reference.setup_inputs()),
returns FULL output [8,128,1,16,9] f32. Internally: data-parallel over the
batch dim across 8 NeuronCores; per core one batch element.

Algebraic reduction (exact):
  X   = conn @ vertices                  (message passing, per batch)
  Y   = mix_nl(cg(X, X))                 (per-node quadratic in X)
  S   = sum_j sph[:, j, :]               (neighbor sum commutes through the
  Z   = mix_rel(cg(Y, S))                 relative-CG stage: x-side is
  out = Z / sqrt(sum Z^2 / 16)            j-independent)

Device pipeline per core — everything node(i)-on-partition. Measured HW
facts baked in: (1) two-input DVE ops lock the shared SBUF port pair, so
GpSimd product offload fully serializes — all products stay on Vector;
(2) DMA never contends with engines, so xbar transposes overlap products
for free; (3) every DMA-completion semaphore costs ~2.1us, so the tail
(last product group + all of stage C) routes through PE transposes.

  A:  Xall[i,288]  = matmul(lhsT=connT, rhs=[vcat|vcat']), bf16, one pass;
                     products read the PSUM accumulator directly (no cast)
  S:  Ssum[i,9]    = 9 scalar-engine ACTIVATE(accum_out) ops over the
                     m-major sph layout (scalar idles during stage B)
  B:  P[i,9984]    = 13 pair products on Vector, bf16 out, packed
                     symmetry-folded slot layout; groups 0..6 -> xbar DMA
                     transpose; group 7 -> PE transposes (4 chunks per
                     PSUM tile via start=False accumulate-onto-cleared)
      Y[i,144]     = 78 bf16 matmuls lhsT=PT-chunk rhs=W2-chunk, 5 s-group
                     PSUM banks (W2 host-folds CG x w_nl, tight-packed)
  C:  per-bank P2 piece = Y_b * Ssum as soon as bank b closes (piece
      layout ordered by close time [2,3,4,0,1]); PE transposes + batched
      copyouts; 14 matmuls lhsT=P2T-chunk rhs=W3-chunk into reused PSUM
Host epilogue: unpack e=(l,c,k) columns, global per-l normalization.
"""
import numpy as np
import ml_dtypes
from math import factorial, sqrt

MAXL = 2
CH = 16
NN = 128
NB = 8
LDIM = [1, 3, 5]
FOFF = [0, 16, 64]
NF = 144
SOFF = [0, 1, 4]

# ------------------------------------------------------------- CG tables
def _cg_coeff(j1, m1, j2, m2, j3, m3):
    if m3 != m1 + m2:
        return 0.0
    pre = sqrt((2 * j3 + 1) * factorial(j3 + j1 - j2) * factorial(j3 - j1 + j2)
               * factorial(j1 + j2 - j3) / factorial(j1 + j2 + j3 + 1))
    pre *= sqrt(factorial(j3 + m3) * factorial(j3 - m3) * factorial(j1 - m1)
                * factorial(j1 + m1) * factorial(j2 - m2) * factorial(j2 + m2))
    s = 0.0
    vmin = max(0, j2 - j3 - m1, j1 - j3 + m2)
    vmax = min(j1 + j2 - j3, j1 - m1, j2 + m2)
    for v in range(vmin, vmax + 1):
        s += (-1) ** v / (factorial(v) * factorial(j1 + j2 - j3 - v)
                          * factorial(j1 - m1 - v) * factorial(j2 + m2 - v)
                          * factorial(j3 - j2 + m1 + v) * factorial(j3 - j1 - m2 + v))
    return pre * s


def _cg_matrix(l1, l2, l):
    M = np.zeros((2 * l1 + 1, 2 * l2 + 1, 2 * l + 1))
    for m1 in range(-l1, l1 + 1):
        for m2 in range(-l2, l2 + 1):
            if -l <= m1 + m2 <= l:
                M[m1 + l1, m2 + l2, m1 + m2 + l] = _cg_coeff(l1, m1, l2, m2, l, m1 + m2)
    return M


def _valid_pairs(l):
    return [(l1, l2) for l1 in range(3) for l2 in range(3)
            if abs(l1 - l2) <= l <= l1 + l2]

# ----------------------------------------------------- packed slot layout
# q = (l1, l2, m1) with l1 <= l2; for diagonal pairs m2 >= m1 (symmetric
# fold: the (m2, m1) ordering's weight folds onto the kept slot with the
# channel grid transposed). Slots of one q are contiguous over its valid,
# contiguous m2-range; each (q, m2) block is a 256-slot (c, d) grid.
def _build_qfold():
    q = []
    off = 0
    for l1 in range(3):
        for l2 in range(l1, 3):
            for m1 in range(2 * l1 + 1):
                mt1 = m1 - l1
                lo = max(0, -2 - mt1 + l2)
                hi = min(2 * l2, 2 - mt1 + l2)
                if l1 == l2:
                    lo = max(lo, m1)
                if lo > hi:
                    continue
                n = hi - lo + 1
                q.append(dict(l1=l1, l2=l2, m1=m1, m2_lo=lo, n_m2=n, off=off))
                off += 256 * n
    return q, off

Q_FOLD, NSLOT = _build_qfold()          # 13 ops, 9984 slots
NCHUNK = NSLOT // 128                   # 78
_QIDX = {(e["l1"], e["l2"], e["m1"]): e for e in Q_FOLD}
# pipeline groups, boundaries chosen to coincide with product-op bounds
# (no op splits). Last group is PE-transposed, the rest go via xbar.
GCHUNKS = [8, 10, 10, 10, 10, 14, 8, 8]
GBOUND = [0]
for c in GCHUNKS:
    GBOUND.append(GBOUND[-1] + 128 * c)
assert GBOUND[-1] == NSLOT
N_XBAR_GROUPS = 6                       # groups 0..5 xbar, rest PE


def _group_ops():
    gops = [[] for _ in GCHUNKS]
    for gi in range(len(GCHUNKS)):
        a, b = GBOUND[gi], GBOUND[gi + 1]
        for qi, e in enumerate(Q_FOLD):
            s0, s1 = e["off"], e["off"] + 256 * e["n_m2"]
            lo, hi = max(a, s0), min(b, s1)
            if lo >= hi:
                continue
            j0 = (lo - s0) // 256
            j1 = (hi - s0) // 256
            gops[gi].append(dict(qi=qi, l1=e["l1"], l2=e["l2"], m1=e["m1"],
                                 m2_lo=e["m2_lo"] + j0, n_m2=j1 - j0, off=lo))
    return gops

G_OPS = _group_ops()

# Y column layout is s-group-major: col = YOFF[g] + (l - |g-2|)*16 + c'.
SG_NCOL = [16, 32, 48, 32, 16]
YOFF = [0, 16, 48, 96, 128]


def _ycol(l, m):
    g = (m - l) + 2
    return YOFF[g] + (l - abs(g - 2)) * 16


def _chunk_meta():
    meta = []
    for e in Q_FOLD:
        mt1 = e["m1"] - e["l1"]
        for j in range(e["n_m2"]):
            g = mt1 + (e["m2_lo"] + j - e["l2"]) + 2
            ncol = 16 * (3 - abs(g - 2))
            for _ in range(2):                      # 2 chunks per 256-block
                meta.append((YOFF[g], ncol, g))
    first, last = {}, {}
    for k, (_, _, g) in enumerate(meta):
        first.setdefault(g, k)
        last[g] = k
    out = []
    woff = 0
    for k, (gc0, ncol, g) in enumerate(meta):
        out.append((gc0, ncol, first[g] == k, last[g] == k, woff))
        woff += ncol
    return out, woff, last

CHUNK_META, NW2T, BANK_LAST = _chunk_meta()

# ---- stage-C P2 piece layout, ordered by bank close time
P2_ORDER = sorted(range(5), key=lambda b: BANK_LAST[b])     # close order
P2_CHUNKS = [-(-9 * SG_NCOL[b] // 128) for b in P2_ORDER]   # per-piece chunks
P2_START = [0]
for c in P2_CHUNKS:
    P2_START.append(P2_START[-1] + c)
NCH3 = P2_START[-1]                                          # 14
NP2PAD = NCH3 * 128

_CAR, _DAR = np.meshgrid(np.arange(16), np.arange(16), indexing="ij")


def _assemble_W2(w_nl):
    """W2[NSLOT, 144] f64: folded CG x w_nl; cols e = FOFF[l]+c'*LDIM[l]+k."""
    W2 = np.zeros((NSLOT, NF))
    for l in range(3):
        off_t = 0
        for (p1, p2) in _valid_pairs(l):
            Cg = _cg_matrix(p1, p2, l)
            wl = np.asarray(w_nl[l], np.float64)
            for m1 in range(2 * p1 + 1):
                for m2 in range(2 * p2 + 1):
                    st = (m1 - p1) + (m2 - p2)
                    if abs(st) > l:
                        continue
                    gc = Cg[m1, m2, st + l]
                    if gc == 0.0:
                        continue
                    if (p1 < p2) or (p1 == p2 and m1 <= m2):
                        e_ = _QIDX[(p1, p2, m1)]
                        base = e_["off"] + (m2 - e_["m2_lo"]) * 256
                        slots = base + _CAR * 16 + _DAR
                    else:
                        e_ = _QIDX[(p2, p1, m2)]
                        base = e_["off"] + (m1 - e_["m2_lo"]) * 256
                        slots = base + _DAR * 16 + _CAR
                    t = off_t + _CAR * 16 + _DAR
                    cols = YOFF[st + 2] + (l - abs(st)) * 16 + np.arange(16)
                    W2[np.ix_(slots.ravel(), cols)] += gc * wl[t.ravel(), :]
            off_t += 256
    return W2


def _assemble_W3(w_rel):
    """W3[NP2PAD, 144]: rows follow the close-ordered P2 piece layout:
    piece for bank b (in P2_ORDER) holds 9*ncol_b live rows (n-major),
    zero-padded to its chunk boundary."""
    W3full = np.zeros((9, NF, NF))       # [n, e, e']
    ar = np.arange(16)
    for l in range(3):
        off_t = 0
        for (p1, p2) in _valid_pairs(l):          # p1 = Y side, p2 = sph side
            Cg = _cg_matrix(p1, p2, l)
            wr = np.asarray(w_rel[l], np.float64)
            for m1 in range(2 * p1 + 1):
                for m2 in range(2 * p2 + 1):
                    st = (m1 - p1) + (m2 - p2)
                    if abs(st) > l:
                        continue
                    gc = Cg[m1, m2, st + l]
                    if gc == 0.0:
                        continue
                    n = SOFF[p2] + m2
                    rows = _ycol(p1, m1) + ar
                    cols = FOFF[l] + ar * LDIM[l] + (st + l)
                    W3full[np.ix_([n], rows, cols)] += gc * wr[off_t:off_t + 16, :][None]
            off_t += 16
    W3 = np.zeros((NP2PAD, NF))
    for pi, b in enumerate(P2_ORDER):
        ncol = SG_NCOL[b]
        base = P2_START[pi] * 128
        for n in range(9):
            W3[base + n * ncol: base + (n + 1) * ncol, :] = \
                W3full[n, YOFF[b]:YOFF[b] + ncol, :]
    return W3

# ------------------------------------------------------------ bass builder
_NC_CACHE = {}


def _build_nc(debug=False):
    import concourse.bacc as bacc
    import concourse.bass as bass
    import concourse.tile as tile
    from concourse import mybir
    from concourse.masks import make_identity

    f32 = mybir.dt.float32
    bf16 = mybir.dt.bfloat16
    nc = bacc.Bacc()
    d_cv = nc.declare_dram_parameter("cv", [128, 128 + 2 * NF], bf16, isOutput=False)
    d_sph = nc.declare_dram_parameter("sph", [128, 9 * 128], bf16, isOutput=False)  # [j, (m,i)]
    d_w2 = nc.declare_dram_parameter("w2", [128, NW2T], bf16, isOutput=False)
    d_w3 = nc.declare_dram_parameter("w3", [128, NCH3 * NF], bf16, isOutput=False)
    d_zout = nc.declare_dram_parameter("zout", [128, NF], f32, isOutput=True)
    if debug:
        d_dbgs = nc.declare_dram_parameter("dbgs", [128, 9], f32, isOutput=True)
        d_dbgp = nc.declare_dram_parameter("dbgp", [128, NSLOT], bf16, isOutput=True)
        d_dbgy = nc.declare_dram_parameter("dbgy", [128, NF], f32, isOutput=True)
        d_dbgp2 = nc.declare_dram_parameter("dbgp2", [128, NP2PAD], bf16, isOutput=True)

    def vap(t, doff, freedims):
        base = t[:] if not isinstance(t, bass.AP) else t
        return bass.AP(tensor=base.tensor, offset=base.offset + doff,
                       ap=[list(base.ap[0])] + [list(d) for d in freedims])

    with tile.TileContext(nc) as tc:
      with (
        tc.tile_pool(name="sb", bufs=1) as sb,
        tc.tile_pool(name="pp", bufs=8) as pp,
        tc.tile_pool(name="ptp", bufs=7) as ptp,
        tc.tile_pool(name="ps_a", bufs=1, space="PSUM") as ps_a,
        tc.tile_pool(name="ps_y", bufs=1, space="PSUM") as ps_y,
        tc.tile_pool(name="ps_t", bufs=2, space="PSUM") as ps_t,
      ):
        # ---- input DMAs. sync: cv then all xbar transposes (transpose
        # crossbar shared unit: single-dispatcher only). scalar: weights+sph.
        cv = sb.tile([128, 128 + 2 * NF], bf16)         # connT | vcat | vcat'
        nc.sync.dma_start(out=cv, in_=d_cv[:, :])
        sph = sb.tile([128, 9, 128], bf16)              # [i, m, j]
        nc.scalar.dma_start(
            out=sph, in_=d_sph[:, :].rearrange("p (m j) -> p m j", m=9, j=128))
        w2 = sb.tile([128, NW2T], bf16)
        wsplit = [0, NW2T // 3, 2 * NW2T // 3, NW2T]
        for g in range(3):
            a, b = wsplit[g], wsplit[g + 1]
            nc.scalar.dma_start(out=vap(w2, a, [[1, b - a]]), in_=d_w2[:, a:b])
        w3 = sb.tile([128, NCH3, NF], bf16)

        # ---- stage A: x_ps[i, 0:144]=X (m-inner), [i,144:288]=X' (c-inner)
        x_ps = ps_a.tile([128, 2 * NF], f32)
        nc.tensor.matmul(x_ps, cv[:, 0:128], cv[:, 128:128 + 2 * NF],
                         start=True, stop=True)
        # stage S: Ssum[i, 9] = one DVE free-axis reduce, before products
        # (sph is dispatched first so its DMA lands by ~10.6)
        ssum = sb.tile([128, 9], f32)
        nc.vector.tensor_reduce(ssum, sph[:, :, :], mybir.AxisListType.X,
                                mybir.AluOpType.add)

        # X' (c-inner half) to SBUF bf16: TensorTensor may read only one
        # input from PSUM, so in1 comes from SBUF while in0 stays in PSUM
        Xc = sb.tile([128, NF], bf16)
        nc.vector.tensor_copy(out=Xc, in_=x_ps[:, NF:2 * NF])

        # ---- ERep[i, qi, (c,16d)] = X[i, FOFF[l1]+m1+c*LDIM[l1]] x16:
        # materialized in0 gives every product op unit innermost strides,
        # which engages the DVE 2x bf16 mode (measured 0.55x per op).
        # Scalar builds it from the PSUM X, chasing ahead of the products.
        erep = sb.tile([128, 13, 256], bf16)
        for qi, e in enumerate(Q_FOLD):
            nc.scalar.copy(
                out=erep[:, qi, :],
                in_=vap(x_ps, FOFF[e["l1"]] + e["m1"],
                        [[LDIM[e["l1"]], 16], [0, 16]]))
        # w3 dispatched after the ERep chase; needed only by stage C
        nc.scalar.dma_start(
            out=w3, in_=d_w3[:, :].rearrange("p (c e) -> p c e", c=NCH3, e=NF))
        ident = sb.tile([128, 128], bf16)
        make_identity(nc, ident)
        # P2 laid out piece-major in close order; pad cols zeroed up front
        P2 = sb.tile([128, NP2PAD], bf16)
        for pi, b in enumerate(P2_ORDER):
            live = 9 * SG_NCOL[b]
            lo, hi = P2_START[pi] * 128 + live, P2_START[pi + 1] * 128
            if hi > lo:
                nc.gpsimd.memset(P2[:, lo:hi], 0.0)
        P2T = sb.tile([128, NCH3, 128], bf16)

        def products(gi):
            gbase = GBOUND[gi]
            gslots = GBOUND[gi + 1] - gbase
            P = pp.tile([128, 2048], bf16)
            for op in G_OPS[gi]:
                l1, l2, m1 = op["l1"], op["l2"], op["m1"]
                nm2 = op["n_m2"]
                nc.vector.tensor_tensor(
                    out=vap(P, op["off"] - gbase,
                            [[256, nm2], [16, 16], [1, 16]]),
                    in0=vap(erep, op["qi"] * 256,
                            [[0, nm2], [16, 16], [1, 16]]),
                    in1=vap(Xc, FOFF[l2] + op["m2_lo"] * 16,
                            [[16, nm2], [0, 16], [1, 16]]),
                    op=mybir.AluOpType.mult)
            return P, gslots

        def ymm(k, PTap):
            gc0, ncol, st_f, sp_f, woff = CHUNK_META[k]
            nc.tensor.matmul(ymixg[YOFF.index(gc0)], PTap,
                             w2[:, woff:woff + ncol], start=st_f, stop=sp_f)

        # PE-transpose a run of chunks of `src` into `dst[:, k0f:...]`,
        # packing `per` chunks per PSUM tile (start=False accumulates onto
        # the bank cleared by the tile's first start=True transpose).
        cp_rr = [0]
        def pe_transpose(src, soff, nch, dst, dchunk0, cp="alt"):
            done = 0
            while done < nch:
                per = min(4, nch - done)
                t_ps = ps_t.tile([128, 512], bf16)
                for j in range(per):
                    nc.tensor.matmul(
                        t_ps[:, j * 128:(j + 1) * 128],
                        src[:, soff + (done + j) * 128: soff + (done + j + 1) * 128],
                        ident, is_transpose=True,
                        start=(j == 0), stop=(j == per - 1),
                        skip_group_check=True)
                c0 = dchunk0 + done
                dstap = vap(dst, c0 * 128, [[1, per * 128]])
                use_s = cp == "s" or (cp == "alt" and cp_rr[0] % 2 == 0)
                if use_s:
                    nc.scalar.copy(out=dstap, in_=t_ps[:, 0:per * 128])
                else:
                    nc.vector.tensor_copy(out=dstap, in_=t_ps[:, 0:per * 128])
                cp_rr[0] += 1
                done += per

        # ---- stage B
        ymixg = [ps_y.tile([128, SG_NCOL[g]], f32, name=f"ymix{g}")
                 for g in range(5)]
        xpt = []
        for gi in range(N_XBAR_GROUPS):
            P, gslots = products(gi)
            nch = gslots // 128
            PT = ptp.tile([128, 16, 128], bf16)
            nc.sync.dma_start(out=PT[:, 0:nch, :], in_=P[:, 0:gslots],
                              transpose=True)
            xpt.append(PT)
            if debug:
                nc.scalar.dma_start(
                    out=d_dbgp[:, GBOUND[gi]:GBOUND[gi] + gslots],
                    in_=P[:, 0:gslots])
        # tail groups via PE transposes (no DMA-completion latency on tail).
        # All ymix matmuls are emitted AFTER every transpose: the PE queue
        # executes in order, and sem-gated xbar matmuls emitted early would
        # head-of-line-block the PE transposes.
        n_pe_chunks = (NSLOT - GBOUND[N_XBAR_GROUPS]) // 128
        PT7 = sb.tile([128, n_pe_chunks, 128], bf16)
        pt7c = 0
        for gi in range(N_XBAR_GROUPS, len(GCHUNKS)):
            P, gslots = products(gi)
            nch = gslots // 128
            last_pe = gi >= N_XBAR_GROUPS + 1
            pe_transpose(P, 0, nch, PT7, pt7c, cp=("v" if last_pe else "s"))
            if debug:
                nc.scalar.dma_start(
                    out=d_dbgp[:, GBOUND[gi]:GBOUND[gi] + gslots],
                    in_=P[:, 0:gslots])
            pt7c += nch
        for gi in range(len(GCHUNKS)):
            nch = (GBOUND[gi + 1] - GBOUND[gi]) // 128
            for c in range(nch):
                k = GBOUND[gi] // 128 + c
                if gi < N_XBAR_GROUPS:
                    ymm(k, xpt[gi][:, c, :])
                else:
                    ymm(k, PT7[:, k - GBOUND[N_XBAR_GROUPS] // 128, :])

        # ---- stage C: per-piece P2 = Y_b * Ssum, PE transposes, 14 matmuls
        for pi, b in enumerate(P2_ORDER):
            ncol = SG_NCOL[b]
            nc.vector.tensor_tensor(
                out=vap(P2, P2_START[pi] * 128, [[ncol, 9], [1, ncol]]),
                in0=vap(ymixg[b], 0, [[0, 9], [1, ncol]]),
                in1=vap(ssum, 0, [[1, 9], [0, ncol]]),
                op=mybir.AluOpType.mult)
            pe_transpose(P2, P2_START[pi] * 128, P2_CHUNKS[pi], P2T,
                         P2_START[pi], cp="alt")
        z_ps = x_ps                     # bank reuse: X consumed by products
        for c in range(NCH3):
            nc.tensor.matmul(z_ps[:, 0:NF], P2T[:, c, :], w3[:, c, :],
                             start=(c == 0), stop=(c == NCH3 - 1))
        zsb = sb.tile([128, NF], f32)
        nc.scalar.activation(zsb, z_ps[:, 0:NF],
                             mybir.ActivationFunctionType.Copy)
        nc.sync.dma_start(out=d_zout[:, :], in_=zsb)

        if debug:
            nc.sync.dma_start(out=d_dbgs[:, :], in_=ssum)
            ydbg = sb.tile([128, NF], f32)
            for g in range(5):
                nc.vector.tensor_copy(
                    out=ydbg[:, YOFF[g]:YOFF[g] + SG_NCOL[g]], in_=ymixg[g])
            nc.sync.dma_start(out=d_dbgy[:, :], in_=ydbg)
            nc.sync.dma_start(out=d_dbgp2[:, :], in_=P2)

    nc.compile()
    return nc

# ------------------------------------------------------------- host entry
LAST_RESULT = {}


def _get_nc():
    if "nc" not in _NC_CACHE:
        _NC_CACHE["nc"] = _build_nc()
    return _NC_CACHE["nc"]


def _pack_w2_tight(W2):
    """[NSLOT, 144] -> [128, NW2T] bf16: per chunk only its live columns."""
    out = np.zeros((128, NW2T))
    for k, (gc0, ncol, _, _, woff) in enumerate(CHUNK_META):
        out[:, woff:woff + ncol] = W2[k * 128:(k + 1) * 128, gc0:gc0 + ncol]
    return out.astype(ml_dtypes.bfloat16)


def _pack_chunked(W, nchunk):
    """[nchunk*128, e] -> [128, nchunk*e] bf16 (chunk-major per partition)."""
    e = W.shape[1]
    return np.ascontiguousarray(
        W.reshape(nchunk, 128, e).transpose(1, 0, 2)
        .astype(ml_dtypes.bfloat16).reshape(128, nchunk * e))


def kernel(vertices_0, vertices_1, vertices_2, connectivity,
           sph_0, sph_1, sph_2,
           w_nl_0, w_nl_1, w_nl_2,
           w_rel_0, w_rel_1, w_rel_2):
    from concourse.bass_utils import run_bass_kernel_spmd

    f = np.float32
    verts = [np.asarray(v, f) for v in (vertices_0, vertices_1, vertices_2)]
    sphs = [np.asarray(s, f) for s in (sph_0, sph_1, sph_2)]
    conn = np.asarray(connectivity)
    W2 = _assemble_W2([np.asarray(w, f) for w in (w_nl_0, w_nl_1, w_nl_2)])
    W3 = _assemble_W3([np.asarray(w, f) for w in (w_rel_0, w_rel_1, w_rel_2)])
    w2p = _pack_w2_tight(W2)
    w3p = _pack_chunked(W3, NCH3)

    in_maps = []
    for b in range(NB):
        vcat = np.concatenate([v[b].reshape(128, -1) for v in verts], axis=1)
        vcat_t = np.concatenate(
            [v[b].reshape(128, CH, LDIM[l]).transpose(0, 2, 1).reshape(128, -1)
             for l, v in enumerate(verts)], axis=1)
        cv = np.concatenate([conn[b].astype(f).T, vcat, vcat_t], axis=1)
        sph_cat = np.concatenate([s[b][:, :, 0, :] for s in sphs], axis=-1)
        sph_mj = sph_cat.transpose(0, 2, 1).reshape(128, 9 * 128)  # [i,(m,j)]
        in_maps.append(dict(
            cv=np.ascontiguousarray(cv.astype(ml_dtypes.bfloat16)),
            sph=np.ascontiguousarray(sph_mj.astype(ml_dtypes.bfloat16)),
            w2=w2p, w3=w3p))

    res = run_bass_kernel_spmd(_get_nc(), in_maps, list(range(NB)))
    LAST_RESULT["res"] = res
    Z = np.stack([res.results[b]["zout"] for b in range(NB)])   # [8, 128, 144]

    # host epilogue: unpack e=(l,c,k) cols, global per-l normalization
    out = np.zeros((NB, 128, 1, 16, 9), dtype=f)
    koff = [0, 1, 4]
    for l in range(3):
        cols = FOFF[l] + (np.arange(16)[:, None] * LDIM[l]
                          + np.arange(LDIM[l])[None, :])
        blk = Z[:, :, cols]                                     # [8,128,16,ld]
        nf = np.sum(blk.astype(np.float64) ** 2)
        out[:, :, 0, :, koff[l]:koff[l] + LDIM[l]] = blk / np.sqrt(nf / 16.0)
    return out


# revision 19
# speedup vs baseline: 1.0491x; 1.0206x over previous
"""Trainium2 Bass kernel for nn_CGLayer (gnn_message_passing).

Contract: kernel(**inputs) takes FULL inputs (as reference.setup_inputs()),
returns FULL output [8,128,1,16,9] f32. Internally: data-parallel over the
batch dim across 8 NeuronCores; per core one batch element.

Algebraic reduction (exact):
  X   = conn @ vertices                  (message passing, per batch)
  Y   = mix_nl(cg(X, X))                 (per-node quadratic in X)
  S   = sum_j sph[:, j, :]               (neighbor sum commutes through the
  Z   = mix_rel(cg(Y, S))                 relative-CG stage: x-side is
  out = Z / sqrt(sum Z^2 / 16)            j-independent)

Device pipeline per core — everything node(i)-on-partition. Measured HW
facts baked in: (1) two-input DVE ops lock the shared SBUF port pair, so
GpSimd product offload fully serializes — all products stay on Vector;
(2) DMA never contends with engines, so xbar transposes overlap products
for free; (3) every DMA-completion semaphore costs ~2.1us, so the tail
(last product group + all of stage C) routes through PE transposes.

  A:  Xall[i,288]  = matmul(lhsT=connT, rhs=[vcat|vcat']), bf16, one pass;
                     products read the PSUM accumulator directly (no cast)
  S:  Ssum[i,9]    = 9 scalar-engine ACTIVATE(accum_out) ops over the
                     m-major sph layout (scalar idles during stage B)
  B:  P[i,9984]    = 13 pair products on Vector, bf16 out, packed
                     symmetry-folded slot layout; groups 0..6 -> xbar DMA
                     transpose; group 7 -> PE transposes (4 chunks per
                     PSUM tile via start=False accumulate-onto-cleared)
      Y[i,144]     = 78 bf16 matmuls lhsT=PT-chunk rhs=W2-chunk, 5 s-group
                     PSUM banks (W2 host-folds CG x w_nl, tight-packed)
  C:  per-bank P2 piece = Y_b * Ssum as soon as bank b closes (piece
      layout ordered by close time [2,3,4,0,1]); PE transposes + batched
      copyouts; 14 matmuls lhsT=P2T-chunk rhs=W3-chunk into reused PSUM
Host epilogue: unpack e=(l,c,k) columns, global per-l normalization.
"""
import numpy as np
import ml_dtypes
from math import factorial, sqrt

MAXL = 2
CH = 16
NN = 128
NB = 8
LDIM = [1, 3, 5]
FOFF = [0, 16, 64]
NF = 144
SOFF = [0, 1, 4]

# ------------------------------------------------------------- CG tables
def _cg_coeff(j1, m1, j2, m2, j3, m3):
    if m3 != m1 + m2:
        return 0.0
    pre = sqrt((2 * j3 + 1) * factorial(j3 + j1 - j2) * factorial(j3 - j1 + j2)
               * factorial(j1 + j2 - j3) / factorial(j1 + j2 + j3 + 1))
    pre *= sqrt(factorial(j3 + m3) * factorial(j3 - m3) * factorial(j1 - m1)
                * factorial(j1 + m1) * factorial(j2 - m2) * factorial(j2 + m2))
    s = 0.0
    vmin = max(0, j2 - j3 - m1, j1 - j3 + m2)
    vmax = min(j1 + j2 - j3, j1 - m1, j2 + m2)
    for v in range(vmin, vmax + 1):
        s += (-1) ** v / (factorial(v) * factorial(j1 + j2 - j3 - v)
                          * factorial(j1 - m1 - v) * factorial(j2 + m2 - v)
                          * factorial(j3 - j2 + m1 + v) * factorial(j3 - j1 - m2 + v))
    return pre * s


def _cg_matrix(l1, l2, l):
    M = np.zeros((2 * l1 + 1, 2 * l2 + 1, 2 * l + 1))
    for m1 in range(-l1, l1 + 1):
        for m2 in range(-l2, l2 + 1):
            if -l <= m1 + m2 <= l:
                M[m1 + l1, m2 + l2, m1 + m2 + l] = _cg_coeff(l1, m1, l2, m2, l, m1 + m2)
    return M


def _valid_pairs(l):
    return [(l1, l2) for l1 in range(3) for l2 in range(3)
            if abs(l1 - l2) <= l <= l1 + l2]

# ----------------------------------------------------- packed slot layout
# q = (l1, l2, m1) with l1 <= l2; for diagonal pairs m2 >= m1 (symmetric
# fold: the (m2, m1) ordering's weight folds onto the kept slot with the
# channel grid transposed). Slots of one q are contiguous over its valid,
# contiguous m2-range; each (q, m2) block is a 256-slot (c, d) grid.
def _build_qfold():
    q = []
    off = 0
    for l1 in range(3):
        for l2 in range(l1, 3):
            for m1 in range(2 * l1 + 1):
                mt1 = m1 - l1
                lo = max(0, -2 - mt1 + l2)
                hi = min(2 * l2, 2 - mt1 + l2)
                if l1 == l2:
                    lo = max(lo, m1)
                if lo > hi:
                    continue
                n = hi - lo + 1
                q.append(dict(l1=l1, l2=l2, m1=m1, m2_lo=lo, n_m2=n, off=off))
                off += 256 * n
    return q, off

Q_FOLD, NSLOT = _build_qfold()          # 13 ops, 9984 slots
NCHUNK = NSLOT // 128                   # 78
_QIDX = {(e["l1"], e["l2"], e["m1"]): e for e in Q_FOLD}
# pipeline groups, boundaries chosen to coincide with product-op bounds
# (no op splits). Last group is PE-transposed, the rest go via xbar.
GCHUNKS = [8, 10, 10, 10, 10, 14, 8, 8]
GBOUND = [0]
for c in GCHUNKS:
    GBOUND.append(GBOUND[-1] + 128 * c)
assert GBOUND[-1] == NSLOT
N_XBAR_GROUPS = 6                       # groups 0..5 xbar, rest PE


def _group_ops():
    gops = [[] for _ in GCHUNKS]
    for gi in range(len(GCHUNKS)):
        a, b = GBOUND[gi], GBOUND[gi + 1]
        for qi, e in enumerate(Q_FOLD):
            s0, s1 = e["off"], e["off"] + 256 * e["n_m2"]
            lo, hi = max(a, s0), min(b, s1)
            if lo >= hi:
                continue
            j0 = (lo - s0) // 256
            j1 = (hi - s0) // 256
            gops[gi].append(dict(qi=qi, l1=e["l1"], l2=e["l2"], m1=e["m1"],
                                 m2_lo=e["m2_lo"] + j0, n_m2=j1 - j0, off=lo))
    return gops

G_OPS = _group_ops()

# Y column layout is s-group-major: col = YOFF[g] + (l - |g-2|)*16 + c'.
SG_NCOL = [16, 32, 48, 32, 16]
YOFF = [0, 16, 48, 96, 128]


def _ycol(l, m):
    g = (m - l) + 2
    return YOFF[g] + (l - abs(g - 2)) * 16


def _chunk_meta():
    meta = []
    for e in Q_FOLD:
        mt1 = e["m1"] - e["l1"]
        for j in range(e["n_m2"]):
            g = mt1 + (e["m2_lo"] + j - e["l2"]) + 2
            ncol = 16 * (3 - abs(g - 2))
            for _ in range(2):                      # 2 chunks per 256-block
                meta.append((YOFF[g], ncol, g))
    first, last = {}, {}
    for k, (_, _, g) in enumerate(meta):
        first.setdefault(g, k)
        last[g] = k
    out = []
    woff = 0
    for k, (gc0, ncol, g) in enumerate(meta):
        out.append((gc0, ncol, first[g] == k, last[g] == k, woff))
        woff += ncol
    return out, woff, last

CHUNK_META, NW2T, BANK_LAST = _chunk_meta()

# ---- stage-C P2 piece layout, ordered by bank close time
P2_ORDER = sorted(range(5), key=lambda b: BANK_LAST[b])     # close order
P2_CHUNKS = [-(-9 * SG_NCOL[b] // 128) for b in P2_ORDER]   # per-piece chunks
P2_START = [0]
for c in P2_CHUNKS:
    P2_START.append(P2_START[-1] + c)
NCH3 = P2_START[-1]                                          # 14
NP2PAD = NCH3 * 128

_CAR, _DAR = np.meshgrid(np.arange(16), np.arange(16), indexing="ij")


def _assemble_W2(w_nl):
    """W2[NSLOT, 144] f64: folded CG x w_nl; cols e = FOFF[l]+c'*LDIM[l]+k."""
    W2 = np.zeros((NSLOT, NF))
    for l in range(3):
        off_t = 0
        for (p1, p2) in _valid_pairs(l):
            Cg = _cg_matrix(p1, p2, l)
            wl = np.asarray(w_nl[l], np.float64)
            for m1 in range(2 * p1 + 1):
                for m2 in range(2 * p2 + 1):
                    st = (m1 - p1) + (m2 - p2)
                    if abs(st) > l:
                        continue
                    gc = Cg[m1, m2, st + l]
                    if gc == 0.0:
                        continue
                    if (p1 < p2) or (p1 == p2 and m1 <= m2):
                        e_ = _QIDX[(p1, p2, m1)]
                        base = e_["off"] + (m2 - e_["m2_lo"]) * 256
                        slots = base + _CAR * 16 + _DAR
                    else:
                        e_ = _QIDX[(p2, p1, m2)]
                        base = e_["off"] + (m1 - e_["m2_lo"]) * 256
                        slots = base + _DAR * 16 + _CAR
                    t = off_t + _CAR * 16 + _DAR
                    cols = YOFF[st + 2] + (l - abs(st)) * 16 + np.arange(16)
                    W2[np.ix_(slots.ravel(), cols)] += gc * wl[t.ravel(), :]
            off_t += 256
    return W2


def _assemble_W3(w_rel):
    """W3[NP2PAD, 144]: rows follow the close-ordered P2 piece layout:
    piece for bank b (in P2_ORDER) holds 9*ncol_b live rows (n-major),
    zero-padded to its chunk boundary."""
    W3full = np.zeros((9, NF, NF))       # [n, e, e']
    ar = np.arange(16)
    for l in range(3):
        off_t = 0
        for (p1, p2) in _valid_pairs(l):          # p1 = Y side, p2 = sph side
            Cg = _cg_matrix(p1, p2, l)
            wr = np.asarray(w_rel[l], np.float64)
            for m1 in range(2 * p1 + 1):
                for m2 in range(2 * p2 + 1):
                    st = (m1 - p1) + (m2 - p2)
                    if abs(st) > l:
                        continue
                    gc = Cg[m1, m2, st + l]
                    if gc == 0.0:
                        continue
                    n = SOFF[p2] + m2
                    rows = _ycol(p1, m1) + ar
                    cols = FOFF[l] + ar * LDIM[l] + (st + l)
                    W3full[np.ix_([n], rows, cols)] += gc * wr[off_t:off_t + 16, :][None]
            off_t += 16
    W3 = np.zeros((NP2PAD, NF))
    for pi, b in enumerate(P2_ORDER):
        ncol = SG_NCOL[b]
        base = P2_START[pi] * 128
        for n in range(9):
            W3[base + n * ncol: base + (n + 1) * ncol, :] = \
                W3full[n, YOFF[b]:YOFF[b] + ncol, :]
    return W3

# ------------------------------------------------------------ bass builder
_NC_CACHE = {}


def _build_nc(debug=False):
    import concourse.bacc as bacc
    import concourse.bass as bass
    import concourse.tile as tile
    from concourse import mybir
    from concourse.masks import make_identity

    f32 = mybir.dt.float32
    bf16 = mybir.dt.bfloat16
    nc = bacc.Bacc()
    d_cv = nc.declare_dram_parameter("cv", [128, 128 + 2 * NF], bf16, isOutput=False)
    d_sph = nc.declare_dram_parameter("sph", [128, 9 * 128], bf16, isOutput=False)  # [j, (m,i)]
    d_w2 = nc.declare_dram_parameter("w2", [128, NW2T], bf16, isOutput=False)
    d_w3 = nc.declare_dram_parameter("w3", [128, NCH3 * NF], bf16, isOutput=False)
    d_zout = nc.declare_dram_parameter("zout", [128, NF], f32, isOutput=True)
    if debug:
        d_dbgs = nc.declare_dram_parameter("dbgs", [128, 9], f32, isOutput=True)
        d_dbgp = nc.declare_dram_parameter("dbgp", [128, NSLOT], bf16, isOutput=True)
        d_dbgy = nc.declare_dram_parameter("dbgy", [128, NF], f32, isOutput=True)
        d_dbgp2 = nc.declare_dram_parameter("dbgp2", [128, NP2PAD], bf16, isOutput=True)

    def vap(t, doff, freedims):
        base = t[:] if not isinstance(t, bass.AP) else t
        return bass.AP(tensor=base.tensor, offset=base.offset + doff,
                       ap=[list(base.ap[0])] + [list(d) for d in freedims])

    with tile.TileContext(nc) as tc:
      with (
        tc.tile_pool(name="sb", bufs=1) as sb,
        tc.tile_pool(name="pp", bufs=8) as pp,
        tc.tile_pool(name="ptp", bufs=7) as ptp,
        tc.tile_pool(name="ps_a", bufs=1, space="PSUM") as ps_a,
        tc.tile_pool(name="ps_y", bufs=1, space="PSUM") as ps_y,
        tc.tile_pool(name="ps_t", bufs=2, space="PSUM") as ps_t,
      ):
        # ---- input DMAs. sync: cv then all xbar transposes (transpose
        # crossbar shared unit: single-dispatcher only). scalar: weights+sph.
        cv = sb.tile([128, 128 + 2 * NF], bf16)         # connT | vcat | vcat'
        nc.sync.dma_start(out=cv, in_=d_cv[:, :])
        sph = sb.tile([128, 9, 128], bf16)              # [i, m, j]
        nc.scalar.dma_start(
            out=sph, in_=d_sph[:, :].rearrange("p (m j) -> p m j", m=9, j=128))
        w2 = sb.tile([128, NW2T], bf16)
        nc.scalar.dma_start(out=w2, in_=d_w2[:, :])
        w3 = sb.tile([128, NCH3, NF], bf16)

        # ---- stage A: x_ps[i, 0:144]=X (m-inner), [i,144:288]=X' (c-inner)
        x_ps = ps_a.tile([128, 2 * NF], f32)
        nc.tensor.matmul(x_ps, cv[:, 0:128], cv[:, 128:128 + 2 * NF],
                         start=True, stop=True)
        # stage S: Ssum[i, 9] = one DVE free-axis reduce, before products
        # (sph is dispatched first so its DMA lands by ~10.6)
        ssum = sb.tile([128, 9], f32)
        nc.vector.tensor_reduce(ssum, sph[:, :, :], mybir.AxisListType.X,
                                mybir.AluOpType.add)

        # X' (c-inner half) to SBUF bf16: TensorTensor may read only one
        # input from PSUM, so in1 comes from SBUF while in0 stays in PSUM
        Xc = sb.tile([128, NF], bf16)
        nc.vector.tensor_copy(out=Xc, in_=x_ps[:, NF:2 * NF])

        # ---- ERep[i, qi, (c,16d)] = X[i, FOFF[l1]+m1+c*LDIM[l1]] x16:
        # materialized in0 gives every product op unit innermost strides,
        # which engages the DVE 2x bf16 mode (measured 0.55x per op).
        # Scalar builds it from the PSUM X, chasing ahead of the products.
        erep = sb.tile([128, 13, 256], bf16)
        for qi, e in enumerate(Q_FOLD):
            nc.scalar.copy(
                out=erep[:, qi, :],
                in_=vap(x_ps, FOFF[e["l1"]] + e["m1"],
                        [[LDIM[e["l1"]], 16], [0, 16]]))
        # w3 dispatched after the ERep chase; needed only by stage C
        nc.scalar.dma_start(
            out=w3, in_=d_w3[:, :].rearrange("p (c e) -> p c e", c=NCH3, e=NF))
        ident = sb.tile([128, 128], bf16)
        make_identity(nc, ident)
        # P2 laid out piece-major in close order; pad cols zeroed up front
        P2 = sb.tile([128, NP2PAD], bf16)
        for pi, b in enumerate(P2_ORDER):
            live = 9 * SG_NCOL[b]
            lo, hi = P2_START[pi] * 128 + live, P2_START[pi + 1] * 128
            if hi > lo:
                nc.gpsimd.memset(P2[:, lo:hi], 0.0)
        P2T = sb.tile([128, NCH3, 128], bf16)

        def products(gi):
            gbase = GBOUND[gi]
            gslots = GBOUND[gi + 1] - gbase
            P = pp.tile([128, 2048], bf16)
            for op in G_OPS[gi]:
                l1, l2, m1 = op["l1"], op["l2"], op["m1"]
                nm2 = op["n_m2"]
                nc.vector.tensor_tensor(
                    out=vap(P, op["off"] - gbase,
                            [[256, nm2], [16, 16], [1, 16]]),
                    in0=vap(erep, op["qi"] * 256,
                            [[0, nm2], [16, 16], [1, 16]]),
                    in1=vap(Xc, FOFF[l2] + op["m2_lo"] * 16,
                            [[16, nm2], [0, 16], [1, 16]]),
                    op=mybir.AluOpType.mult)
            return P, gslots

        def ymm(k, PTap):
            gc0, ncol, st_f, sp_f, woff = CHUNK_META[k]
            nc.tensor.matmul(ymixg[YOFF.index(gc0)], PTap,
                             w2[:, woff:woff + ncol], start=st_f, stop=sp_f)

        # PE-transpose a run of chunks of `src` into `dst[:, k0f:...]`,
        # packing `per` chunks per PSUM tile (start=False accumulates onto
        # the bank cleared by the tile's first start=True transpose).
        cp_rr = [0]
        def pe_transpose(src, soff, nch, dst, dchunk0, cp="alt"):
            done = 0
            while done < nch:
                per = min(4, nch - done)
                t_ps = ps_t.tile([128, 512], bf16)
                for j in range(per):
                    nc.tensor.matmul(
                        t_ps[:, j * 128:(j + 1) * 128],
                        src[:, soff + (done + j) * 128: soff + (done + j + 1) * 128],
                        ident, is_transpose=True,
                        start=(j == 0), stop=(j == per - 1),
                        skip_group_check=True)
                c0 = dchunk0 + done
                dstap = vap(dst, c0 * 128, [[1, per * 128]])
                use_s = cp == "s" or (cp == "alt" and cp_rr[0] % 2 == 0)
                if use_s:
                    nc.scalar.copy(out=dstap, in_=t_ps[:, 0:per * 128])
                else:
                    nc.vector.tensor_copy(out=dstap, in_=t_ps[:, 0:per * 128])
                cp_rr[0] += 1
                done += per

        # ---- stage B
        ymixg = [ps_y.tile([128, SG_NCOL[g]], f32, name=f"ymix{g}")
                 for g in range(5)]
        xpt = []
        for gi in range(N_XBAR_GROUPS):
            P, gslots = products(gi)
            nch = gslots // 128
            PT = ptp.tile([128, 16, 128], bf16)
            nc.sync.dma_start(out=PT[:, 0:nch, :], in_=P[:, 0:gslots],
                              transpose=True)
            xpt.append(PT)
            if debug:
                nc.scalar.dma_start(
                    out=d_dbgp[:, GBOUND[gi]:GBOUND[gi] + gslots],
                    in_=P[:, 0:gslots])
        # tail groups via PE transposes (no DMA-completion latency on tail).
        # All ymix matmuls are emitted AFTER every transpose: the PE queue
        # executes in order, and sem-gated xbar matmuls emitted early would
        # head-of-line-block the PE transposes.
        n_pe_chunks = (NSLOT - GBOUND[N_XBAR_GROUPS]) // 128
        PT7 = sb.tile([128, n_pe_chunks, 128], bf16)
        pt7c = 0
        for gi in range(N_XBAR_GROUPS, len(GCHUNKS)):
            P, gslots = products(gi)
            nch = gslots // 128
            last_pe = gi >= N_XBAR_GROUPS + 1
            pe_transpose(P, 0, nch, PT7, pt7c, cp=("v" if last_pe else "s"))
            if debug:
                nc.scalar.dma_start(
                    out=d_dbgp[:, GBOUND[gi]:GBOUND[gi] + gslots],
                    in_=P[:, 0:gslots])
            pt7c += nch
        for gi in range(len(GCHUNKS)):
            nch = (GBOUND[gi + 1] - GBOUND[gi]) // 128
            for c in range(nch):
                k = GBOUND[gi] // 128 + c
                if gi < N_XBAR_GROUPS:
                    ymm(k, xpt[gi][:, c, :])
                else:
                    ymm(k, PT7[:, k - GBOUND[N_XBAR_GROUPS] // 128, :])

        # ---- stage C: per-piece P2 = Y_b * Ssum, PE transposes, 14 matmuls
        for pi, b in enumerate(P2_ORDER):
            ncol = SG_NCOL[b]
            nc.vector.tensor_tensor(
                out=vap(P2, P2_START[pi] * 128, [[ncol, 9], [1, ncol]]),
                in0=vap(ymixg[b], 0, [[0, 9], [1, ncol]]),
                in1=vap(ssum, 0, [[1, 9], [0, ncol]]),
                op=mybir.AluOpType.mult)
            pe_transpose(P2, P2_START[pi] * 128, P2_CHUNKS[pi], P2T,
                         P2_START[pi], cp="alt")
        z_ps = x_ps                     # bank reuse: X consumed by products
        for c in range(NCH3):
            nc.tensor.matmul(z_ps[:, 0:NF], P2T[:, c, :], w3[:, c, :],
                             start=(c == 0), stop=(c == NCH3 - 1))
        zsb = sb.tile([128, NF], f32)
        nc.scalar.activation(zsb, z_ps[:, 0:NF],
                             mybir.ActivationFunctionType.Copy)
        nc.sync.dma_start(out=d_zout[:, :], in_=zsb)

        if debug:
            nc.sync.dma_start(out=d_dbgs[:, :], in_=ssum)
            ydbg = sb.tile([128, NF], f32)
            for g in range(5):
                nc.vector.tensor_copy(
                    out=ydbg[:, YOFF[g]:YOFF[g] + SG_NCOL[g]], in_=ymixg[g])
            nc.sync.dma_start(out=d_dbgy[:, :], in_=ydbg)
            nc.sync.dma_start(out=d_dbgp2[:, :], in_=P2)

    nc.compile()
    return nc

# ------------------------------------------------------------- host entry
LAST_RESULT = {}


def _get_nc():
    if "nc" not in _NC_CACHE:
        _NC_CACHE["nc"] = _build_nc()
    return _NC_CACHE["nc"]


def _pack_w2_tight(W2):
    """[NSLOT, 144] -> [128, NW2T] bf16: per chunk only its live columns."""
    out = np.zeros((128, NW2T))
    for k, (gc0, ncol, _, _, woff) in enumerate(CHUNK_META):
        out[:, woff:woff + ncol] = W2[k * 128:(k + 1) * 128, gc0:gc0 + ncol]
    return out.astype(ml_dtypes.bfloat16)


def _pack_chunked(W, nchunk):
    """[nchunk*128, e] -> [128, nchunk*e] bf16 (chunk-major per partition)."""
    e = W.shape[1]
    return np.ascontiguousarray(
        W.reshape(nchunk, 128, e).transpose(1, 0, 2)
        .astype(ml_dtypes.bfloat16).reshape(128, nchunk * e))


def kernel(vertices_0, vertices_1, vertices_2, connectivity,
           sph_0, sph_1, sph_2,
           w_nl_0, w_nl_1, w_nl_2,
           w_rel_0, w_rel_1, w_rel_2):
    from concourse.bass_utils import run_bass_kernel_spmd

    f = np.float32
    verts = [np.asarray(v, f) for v in (vertices_0, vertices_1, vertices_2)]
    sphs = [np.asarray(s, f) for s in (sph_0, sph_1, sph_2)]
    conn = np.asarray(connectivity)
    W2 = _assemble_W2([np.asarray(w, f) for w in (w_nl_0, w_nl_1, w_nl_2)])
    W3 = _assemble_W3([np.asarray(w, f) for w in (w_rel_0, w_rel_1, w_rel_2)])
    w2p = _pack_w2_tight(W2)
    w3p = _pack_chunked(W3, NCH3)

    in_maps = []
    for b in range(NB):
        vcat = np.concatenate([v[b].reshape(128, -1) for v in verts], axis=1)
        vcat_t = np.concatenate(
            [v[b].reshape(128, CH, LDIM[l]).transpose(0, 2, 1).reshape(128, -1)
             for l, v in enumerate(verts)], axis=1)
        cv = np.concatenate([conn[b].astype(f).T, vcat, vcat_t], axis=1)
        sph_cat = np.concatenate([s[b][:, :, 0, :] for s in sphs], axis=-1)
        sph_mj = sph_cat.transpose(0, 2, 1).reshape(128, 9 * 128)  # [i,(m,j)]
        in_maps.append(dict(
            cv=np.ascontiguousarray(cv.astype(ml_dtypes.bfloat16)),
            sph=np.ascontiguousarray(sph_mj.astype(ml_dtypes.bfloat16)),
            w2=w2p, w3=w3p))

    res = run_bass_kernel_spmd(_get_nc(), in_maps, list(range(NB)))
    LAST_RESULT["res"] = res
    Z = np.stack([res.results[b]["zout"] for b in range(NB)])   # [8, 128, 144]

    # host epilogue: unpack e=(l,c,k) cols, global per-l normalization
    out = np.zeros((NB, 128, 1, 16, 9), dtype=f)
    koff = [0, 1, 4]
    for l in range(3):
        cols = FOFF[l] + (np.arange(16)[:, None] * LDIM[l]
                          + np.arange(LDIM[l])[None, :])
        blk = Z[:, :, cols]                                     # [8,128,16,ld]
        nf = np.sum(blk.astype(np.float64) ** 2)
        out[:, :, 0, :, koff[l]:koff[l] + LDIM[l]] = blk / np.sqrt(nf / 16.0)
    return out


# revision 20
# speedup vs baseline: 1.0524x; 1.0032x over previous
"""Trainium2 Bass kernel for nn_CGLayer (gnn_message_passing).

Contract: kernel(**inputs) takes FULL inputs (as reference.setup_inputs()),
returns FULL output [8,128,1,16,9] f32. Internally: data-parallel over the
batch dim across 8 NeuronCores; per core one batch element.

Algebraic reduction (exact):
  X   = conn @ vertices                  (message passing, per batch)
  Y   = mix_nl(cg(X, X))                 (per-node quadratic in X)
  S   = sum_j sph[:, j, :]               (neighbor sum commutes through the
  Z   = mix_rel(cg(Y, S))                 relative-CG stage: x-side is
  out = Z / sqrt(sum Z^2 / 16)            j-independent)

Device pipeline per core — everything node(i)-on-partition. Measured HW
facts baked in: (1) two-input DVE ops lock the shared SBUF port pair, so
GpSimd product offload fully serializes — all products stay on Vector;
(2) DMA never contends with engines, so xbar transposes overlap products
for free; (3) every DMA-completion semaphore costs ~2.1us, so the tail
(last product group + all of stage C) routes through PE transposes.

  A:  Xall[i,288]  = matmul(lhsT=connT, rhs=[vcat|vcat']), bf16, one pass;
                     products read the PSUM accumulator directly (no cast)
  S:  Ssum[i,9]    = 9 scalar-engine ACTIVATE(accum_out) ops over the
                     m-major sph layout (scalar idles during stage B)
  B:  P[i,9984]    = 13 pair products on Vector, bf16 out, packed
                     symmetry-folded slot layout; groups 0..6 -> xbar DMA
                     transpose; group 7 -> PE transposes (4 chunks per
                     PSUM tile via start=False accumulate-onto-cleared)
      Y[i,144]     = 78 bf16 matmuls lhsT=PT-chunk rhs=W2-chunk, 5 s-group
                     PSUM banks (W2 host-folds CG x w_nl, tight-packed)
  C:  per-bank P2 piece = Y_b * Ssum as soon as bank b closes (piece
      layout ordered by close time [2,3,4,0,1]); PE transposes + batched
      copyouts; 14 matmuls lhsT=P2T-chunk rhs=W3-chunk into reused PSUM
Host epilogue: unpack e=(l,c,k) columns, global per-l normalization.
"""
import numpy as np
import ml_dtypes
from math import factorial, sqrt

MAXL = 2
CH = 16
NN = 128
NB = 8
LDIM = [1, 3, 5]
FOFF = [0, 16, 64]
NF = 144
SOFF = [0, 1, 4]

# ------------------------------------------------------------- CG tables
def _cg_coeff(j1, m1, j2, m2, j3, m3):
    if m3 != m1 + m2:
        return 0.0
    pre = sqrt((2 * j3 + 1) * factorial(j3 + j1 - j2) * factorial(j3 - j1 + j2)
               * factorial(j1 + j2 - j3) / factorial(j1 + j2 + j3 + 1))
    pre *= sqrt(factorial(j3 + m3) * factorial(j3 - m3) * factorial(j1 - m1)
                * factorial(j1 + m1) * factorial(j2 - m2) * factorial(j2 + m2))
    s = 0.0
    vmin = max(0, j2 - j3 - m1, j1 - j3 + m2)
    vmax = min(j1 + j2 - j3, j1 - m1, j2 + m2)
    for v in range(vmin, vmax + 1):
        s += (-1) ** v / (factorial(v) * factorial(j1 + j2 - j3 - v)
                          * factorial(j1 - m1 - v) * factorial(j2 + m2 - v)
                          * factorial(j3 - j2 + m1 + v) * factorial(j3 - j1 - m2 + v))
    return pre * s


def _cg_matrix(l1, l2, l):
    M = np.zeros((2 * l1 + 1, 2 * l2 + 1, 2 * l + 1))
    for m1 in range(-l1, l1 + 1):
        for m2 in range(-l2, l2 + 1):
            if -l <= m1 + m2 <= l:
                M[m1 + l1, m2 + l2, m1 + m2 + l] = _cg_coeff(l1, m1, l2, m2, l, m1 + m2)
    return M


def _valid_pairs(l):
    return [(l1, l2) for l1 in range(3) for l2 in range(3)
            if abs(l1 - l2) <= l <= l1 + l2]

# ----------------------------------------------------- packed slot layout
# q = (l1, l2, m1) with l1 <= l2; for diagonal pairs m2 >= m1 (symmetric
# fold: the (m2, m1) ordering's weight folds onto the kept slot with the
# channel grid transposed). Slots of one q are contiguous over its valid,
# contiguous m2-range; each (q, m2) block is a 256-slot (c, d) grid.
def _build_qfold():
    q = []
    off = 0
    for l1 in range(3):
        for l2 in range(l1, 3):
            for m1 in range(2 * l1 + 1):
                mt1 = m1 - l1
                lo = max(0, -2 - mt1 + l2)
                hi = min(2 * l2, 2 - mt1 + l2)
                if l1 == l2:
                    lo = max(lo, m1)
                if lo > hi:
                    continue
                n = hi - lo + 1
                q.append(dict(l1=l1, l2=l2, m1=m1, m2_lo=lo, n_m2=n, off=off))
                off += 256 * n
    return q, off

Q_FOLD, NSLOT = _build_qfold()          # 13 ops, 9984 slots
NCHUNK = NSLOT // 128                   # 78
_QIDX = {(e["l1"], e["l2"], e["m1"]): e for e in Q_FOLD}
# pipeline groups, boundaries chosen to coincide with product-op bounds
# (no op splits). Last group is PE-transposed, the rest go via xbar.
GCHUNKS = [8, 10, 10, 10, 10, 8, 6, 8, 8]
GBOUND = [0]
for c in GCHUNKS:
    GBOUND.append(GBOUND[-1] + 128 * c)
assert GBOUND[-1] == NSLOT
N_XBAR_GROUPS = 5                       # groups 0..4 xbar, rest PE


def _group_ops():
    gops = [[] for _ in GCHUNKS]
    for gi in range(len(GCHUNKS)):
        a, b = GBOUND[gi], GBOUND[gi + 1]
        for qi, e in enumerate(Q_FOLD):
            s0, s1 = e["off"], e["off"] + 256 * e["n_m2"]
            lo, hi = max(a, s0), min(b, s1)
            if lo >= hi:
                continue
            j0 = (lo - s0) // 256
            j1 = (hi - s0) // 256
            gops[gi].append(dict(qi=qi, l1=e["l1"], l2=e["l2"], m1=e["m1"],
                                 m2_lo=e["m2_lo"] + j0, n_m2=j1 - j0, off=lo))
    return gops

G_OPS = _group_ops()

# Y column layout is s-group-major: col = YOFF[g] + (l - |g-2|)*16 + c'.
SG_NCOL = [16, 32, 48, 32, 16]
YOFF = [0, 16, 48, 96, 128]


def _ycol(l, m):
    g = (m - l) + 2
    return YOFF[g] + (l - abs(g - 2)) * 16


def _chunk_meta():
    meta = []
    for e in Q_FOLD:
        mt1 = e["m1"] - e["l1"]
        for j in range(e["n_m2"]):
            g = mt1 + (e["m2_lo"] + j - e["l2"]) + 2
            ncol = 16 * (3 - abs(g - 2))
            for _ in range(2):                      # 2 chunks per 256-block
                meta.append((YOFF[g], ncol, g))
    first, last = {}, {}
    for k, (_, _, g) in enumerate(meta):
        first.setdefault(g, k)
        last[g] = k
    out = []
    woff = 0
    for k, (gc0, ncol, g) in enumerate(meta):
        out.append((gc0, ncol, first[g] == k, last[g] == k, woff))
        woff += ncol
    return out, woff, last

CHUNK_META, NW2T, BANK_LAST = _chunk_meta()

# ---- stage-C P2 piece layout, ordered by bank close time
P2_ORDER = sorted(range(5), key=lambda b: BANK_LAST[b])     # close order
P2_CHUNKS = [-(-9 * SG_NCOL[b] // 128) for b in P2_ORDER]   # per-piece chunks
P2_START = [0]
for c in P2_CHUNKS:
    P2_START.append(P2_START[-1] + c)
NCH3 = P2_START[-1]                                          # 14
NP2PAD = NCH3 * 128

_CAR, _DAR = np.meshgrid(np.arange(16), np.arange(16), indexing="ij")


def _assemble_W2(w_nl):
    """W2[NSLOT, 144] f64: folded CG x w_nl; cols e = FOFF[l]+c'*LDIM[l]+k."""
    W2 = np.zeros((NSLOT, NF))
    for l in range(3):
        off_t = 0
        for (p1, p2) in _valid_pairs(l):
            Cg = _cg_matrix(p1, p2, l)
            wl = np.asarray(w_nl[l], np.float64)
            for m1 in range(2 * p1 + 1):
                for m2 in range(2 * p2 + 1):
                    st = (m1 - p1) + (m2 - p2)
                    if abs(st) > l:
                        continue
                    gc = Cg[m1, m2, st + l]
                    if gc == 0.0:
                        continue
                    if (p1 < p2) or (p1 == p2 and m1 <= m2):
                        e_ = _QIDX[(p1, p2, m1)]
                        base = e_["off"] + (m2 - e_["m2_lo"]) * 256
                        slots = base + _CAR * 16 + _DAR
                    else:
                        e_ = _QIDX[(p2, p1, m2)]
                        base = e_["off"] + (m1 - e_["m2_lo"]) * 256
                        slots = base + _DAR * 16 + _CAR
                    t = off_t + _CAR * 16 + _DAR
                    cols = YOFF[st + 2] + (l - abs(st)) * 16 + np.arange(16)
                    W2[np.ix_(slots.ravel(), cols)] += gc * wl[t.ravel(), :]
            off_t += 256
    return W2


def _assemble_W3(w_rel):
    """W3[NP2PAD, 144]: rows follow the close-ordered P2 piece layout:
    piece for bank b (in P2_ORDER) holds 9*ncol_b live rows (n-major),
    zero-padded to its chunk boundary."""
    W3full = np.zeros((9, NF, NF))       # [n, e, e']
    ar = np.arange(16)
    for l in range(3):
        off_t = 0
        for (p1, p2) in _valid_pairs(l):          # p1 = Y side, p2 = sph side
            Cg = _cg_matrix(p1, p2, l)
            wr = np.asarray(w_rel[l], np.float64)
            for m1 in range(2 * p1 + 1):
                for m2 in range(2 * p2 + 1):
                    st = (m1 - p1) + (m2 - p2)
                    if abs(st) > l:
                        continue
                    gc = Cg[m1, m2, st + l]
                    if gc == 0.0:
                        continue
                    n = SOFF[p2] + m2
                    rows = _ycol(p1, m1) + ar
                    cols = FOFF[l] + ar * LDIM[l] + (st + l)
                    W3full[np.ix_([n], rows, cols)] += gc * wr[off_t:off_t + 16, :][None]
            off_t += 16
    W3 = np.zeros((NP2PAD, NF))
    for pi, b in enumerate(P2_ORDER):
        ncol = SG_NCOL[b]
        base = P2_START[pi] * 128
        for n in range(9):
            W3[base + n * ncol: base + (n + 1) * ncol, :] = \
                W3full[n, YOFF[b]:YOFF[b] + ncol, :]
    return W3

# ------------------------------------------------------------ bass builder
_NC_CACHE = {}


def _build_nc(debug=False):
    import concourse.bacc as bacc
    import concourse.bass as bass
    import concourse.tile as tile
    from concourse import mybir
    from concourse.masks import make_identity

    f32 = mybir.dt.float32
    bf16 = mybir.dt.bfloat16
    nc = bacc.Bacc()
    d_cv = nc.declare_dram_parameter("cv", [128, 128 + 2 * NF], bf16, isOutput=False)
    d_sph = nc.declare_dram_parameter("sph", [128, 9 * 128], bf16, isOutput=False)  # [j, (m,i)]
    d_w2 = nc.declare_dram_parameter("w2", [128, NW2T], bf16, isOutput=False)
    d_w3 = nc.declare_dram_parameter("w3", [128, NCH3 * NF], bf16, isOutput=False)
    d_zout = nc.declare_dram_parameter("zout", [128, NF], f32, isOutput=True)
    if debug:
        d_dbgs = nc.declare_dram_parameter("dbgs", [128, 9], f32, isOutput=True)
        d_dbgp = nc.declare_dram_parameter("dbgp", [128, NSLOT], bf16, isOutput=True)
        d_dbgy = nc.declare_dram_parameter("dbgy", [128, NF], f32, isOutput=True)
        d_dbgp2 = nc.declare_dram_parameter("dbgp2", [128, NP2PAD], bf16, isOutput=True)

    def vap(t, doff, freedims):
        base = t[:] if not isinstance(t, bass.AP) else t
        return bass.AP(tensor=base.tensor, offset=base.offset + doff,
                       ap=[list(base.ap[0])] + [list(d) for d in freedims])

    with tile.TileContext(nc) as tc:
      with (
        tc.tile_pool(name="sb", bufs=1) as sb,
        tc.tile_pool(name="pp", bufs=8) as pp,
        tc.tile_pool(name="ptp", bufs=7) as ptp,
        tc.tile_pool(name="ps_a", bufs=1, space="PSUM") as ps_a,
        tc.tile_pool(name="ps_y", bufs=1, space="PSUM") as ps_y,
        tc.tile_pool(name="ps_t", bufs=2, space="PSUM") as ps_t,
      ):
        # ---- input DMAs. sync: cv then all xbar transposes (transpose
        # crossbar shared unit: single-dispatcher only). scalar: weights+sph.
        cv = sb.tile([128, 128 + 2 * NF], bf16)         # connT | vcat | vcat'
        nc.sync.dma_start(out=cv, in_=d_cv[:, :])
        sph = sb.tile([128, 9, 128], bf16)              # [i, m, j]
        nc.scalar.dma_start(
            out=sph, in_=d_sph[:, :].rearrange("p (m j) -> p m j", m=9, j=128))
        w2 = sb.tile([128, NW2T], bf16)
        nc.scalar.dma_start(out=w2, in_=d_w2[:, :])
        w3 = sb.tile([128, NCH3, NF], bf16)

        # ---- stage A: x_ps[i, 0:144]=X (m-inner), [i,144:288]=X' (c-inner)
        x_ps = ps_a.tile([128, 2 * NF], f32)
        nc.tensor.matmul(x_ps, cv[:, 0:128], cv[:, 128:128 + 2 * NF],
                         start=True, stop=True)
        # stage S: Ssum[i, 9] = one DVE free-axis reduce, before products
        # (sph is dispatched first so its DMA lands by ~10.6)
        ssum = sb.tile([128, 9], f32)
        nc.vector.tensor_reduce(ssum, sph[:, :, :], mybir.AxisListType.X,
                                mybir.AluOpType.add)

        # X' (c-inner half) to SBUF bf16: TensorTensor may read only one
        # input from PSUM, so in1 comes from SBUF while in0 stays in PSUM
        Xc = sb.tile([128, NF], bf16)
        nc.vector.tensor_copy(out=Xc, in_=x_ps[:, NF:2 * NF])

        # ---- ERep[i, qi, (c,16d)] = X[i, FOFF[l1]+m1+c*LDIM[l1]] x16:
        # materialized in0 gives every product op unit innermost strides,
        # which engages the DVE 2x bf16 mode (measured 0.55x per op).
        # Scalar builds it from the PSUM X, chasing ahead of the products.
        erep = sb.tile([128, 13, 256], bf16)
        for qi, e in enumerate(Q_FOLD):
            nc.scalar.copy(
                out=erep[:, qi, :],
                in_=vap(x_ps, FOFF[e["l1"]] + e["m1"],
                        [[LDIM[e["l1"]], 16], [0, 16]]))
        # w3 dispatched after the ERep chase; needed only by stage C
        nc.scalar.dma_start(
            out=w3, in_=d_w3[:, :].rearrange("p (c e) -> p c e", c=NCH3, e=NF))
        ident = sb.tile([128, 128], bf16)
        make_identity(nc, ident)
        # P2 laid out piece-major in close order; pad cols zeroed up front
        P2 = sb.tile([128, NP2PAD], bf16)
        for pi, b in enumerate(P2_ORDER):
            live = 9 * SG_NCOL[b]
            lo, hi = P2_START[pi] * 128 + live, P2_START[pi + 1] * 128
            if hi > lo:
                nc.gpsimd.memset(P2[:, lo:hi], 0.0)
        P2T = sb.tile([128, NCH3, 128], bf16)

        def products(gi):
            gbase = GBOUND[gi]
            gslots = GBOUND[gi + 1] - gbase
            P = pp.tile([128, 2048], bf16)
            for op in G_OPS[gi]:
                l1, l2, m1 = op["l1"], op["l2"], op["m1"]
                nm2 = op["n_m2"]
                nc.vector.tensor_tensor(
                    out=vap(P, op["off"] - gbase,
                            [[256, nm2], [16, 16], [1, 16]]),
                    in0=vap(erep, op["qi"] * 256,
                            [[0, nm2], [16, 16], [1, 16]]),
                    in1=vap(Xc, FOFF[l2] + op["m2_lo"] * 16,
                            [[16, nm2], [0, 16], [1, 16]]),
                    op=mybir.AluOpType.mult)
            return P, gslots

        def ymm(k, PTap):
            gc0, ncol, st_f, sp_f, woff = CHUNK_META[k]
            nc.tensor.matmul(ymixg[YOFF.index(gc0)], PTap,
                             w2[:, woff:woff + ncol], start=st_f, stop=sp_f)

        # PE-transpose a run of chunks of `src` into `dst[:, k0f:...]`,
        # packing `per` chunks per PSUM tile (start=False accumulates onto
        # the bank cleared by the tile's first start=True transpose).
        cp_rr = [0]
        def pe_transpose(src, soff, nch, dst, dchunk0, cp="alt"):
            done = 0
            while done < nch:
                per = min(4, nch - done)
                t_ps = ps_t.tile([128, 512], bf16)
                for j in range(per):
                    nc.tensor.matmul(
                        t_ps[:, j * 128:(j + 1) * 128],
                        src[:, soff + (done + j) * 128: soff + (done + j + 1) * 128],
                        ident, is_transpose=True,
                        start=(j == 0), stop=(j == per - 1),
                        skip_group_check=True)
                c0 = dchunk0 + done
                dstap = vap(dst, c0 * 128, [[1, per * 128]])
                use_s = cp == "s" or (cp == "alt" and cp_rr[0] % 2 == 0)
                if use_s:
                    nc.scalar.copy(out=dstap, in_=t_ps[:, 0:per * 128])
                else:
                    nc.vector.tensor_copy(out=dstap, in_=t_ps[:, 0:per * 128])
                cp_rr[0] += 1
                done += per

        # ---- stage B
        ymixg = [ps_y.tile([128, SG_NCOL[g]], f32, name=f"ymix{g}")
                 for g in range(5)]
        xpt = []
        for gi in range(N_XBAR_GROUPS):
            P, gslots = products(gi)
            nch = gslots // 128
            PT = ptp.tile([128, 16, 128], bf16)
            nc.sync.dma_start(out=PT[:, 0:nch, :], in_=P[:, 0:gslots],
                              transpose=True)
            xpt.append(PT)
            if debug:
                nc.scalar.dma_start(
                    out=d_dbgp[:, GBOUND[gi]:GBOUND[gi] + gslots],
                    in_=P[:, 0:gslots])
        # tail groups via PE transposes (no DMA-completion latency on tail).
        # All ymix matmuls are emitted AFTER every transpose: the PE queue
        # executes in order, and sem-gated xbar matmuls emitted early would
        # head-of-line-block the PE transposes.
        n_pe_chunks = (NSLOT - GBOUND[N_XBAR_GROUPS]) // 128
        PT7 = sb.tile([128, n_pe_chunks, 128], bf16)
        pt7c = 0
        for gi in range(N_XBAR_GROUPS, len(GCHUNKS)):
            P, gslots = products(gi)
            nch = gslots // 128
            last_pe = gi >= N_XBAR_GROUPS + 2
            pe_transpose(P, 0, nch, PT7, pt7c, cp=("v" if last_pe else "s"))
            if debug:
                nc.scalar.dma_start(
                    out=d_dbgp[:, GBOUND[gi]:GBOUND[gi] + gslots],
                    in_=P[:, 0:gslots])
            pt7c += nch
        for gi in range(len(GCHUNKS)):
            nch = (GBOUND[gi + 1] - GBOUND[gi]) // 128
            for c in range(nch):
                k = GBOUND[gi] // 128 + c
                if gi < N_XBAR_GROUPS:
                    ymm(k, xpt[gi][:, c, :])
                else:
                    ymm(k, PT7[:, k - GBOUND[N_XBAR_GROUPS] // 128, :])

        # ---- stage C: per-piece P2 = Y_b * Ssum, PE transposes, 14 matmuls
        for pi, b in enumerate(P2_ORDER):
            ncol = SG_NCOL[b]
            nc.vector.tensor_tensor(
                out=vap(P2, P2_START[pi] * 128, [[ncol, 9], [1, ncol]]),
                in0=vap(ymixg[b], 0, [[0, 9], [1, ncol]]),
                in1=vap(ssum, 0, [[1, 9], [0, ncol]]),
                op=mybir.AluOpType.mult)
            pe_transpose(P2, P2_START[pi] * 128, P2_CHUNKS[pi], P2T,
                         P2_START[pi], cp="alt")
        z_ps = x_ps                     # bank reuse: X consumed by products
        for c in range(NCH3):
            nc.tensor.matmul(z_ps[:, 0:NF], P2T[:, c, :], w3[:, c, :],
                             start=(c == 0), stop=(c == NCH3 - 1))
        zsb = sb.tile([128, NF], f32)
        nc.scalar.activation(zsb, z_ps[:, 0:NF],
                             mybir.ActivationFunctionType.Copy)
        nc.sync.dma_start(out=d_zout[:, :], in_=zsb)

        if debug:
            nc.sync.dma_start(out=d_dbgs[:, :], in_=ssum)
            ydbg = sb.tile([128, NF], f32)
            for g in range(5):
                nc.vector.tensor_copy(
                    out=ydbg[:, YOFF[g]:YOFF[g] + SG_NCOL[g]], in_=ymixg[g])
            nc.sync.dma_start(out=d_dbgy[:, :], in_=ydbg)
            nc.sync.dma_start(out=d_dbgp2[:, :], in_=P2)

    nc.compile()
    return nc

# ------------------------------------------------------------- host entry
LAST_RESULT = {}


def _get_nc():
    if "nc" not in _NC_CACHE:
        _NC_CACHE["nc"] = _build_nc()
    return _NC_CACHE["nc"]


def _pack_w2_tight(W2):
    """[NSLOT, 144] -> [128, NW2T] bf16: per chunk only its live columns."""
    out = np.zeros((128, NW2T))
    for k, (gc0, ncol, _, _, woff) in enumerate(CHUNK_META):
        out[:, woff:woff + ncol] = W2[k * 128:(k + 1) * 128, gc0:gc0 + ncol]
    return out.astype(ml_dtypes.bfloat16)


def _pack_chunked(W, nchunk):
    """[nchunk*128, e] -> [128, nchunk*e] bf16 (chunk-major per partition)."""
    e = W.shape[1]
    return np.ascontiguousarray(
        W.reshape(nchunk, 128, e).transpose(1, 0, 2)
        .astype(ml_dtypes.bfloat16).reshape(128, nchunk * e))


def kernel(vertices_0, vertices_1, vertices_2, connectivity,
           sph_0, sph_1, sph_2,
           w_nl_0, w_nl_1, w_nl_2,
           w_rel_0, w_rel_1, w_rel_2):
    from concourse.bass_utils import run_bass_kernel_spmd

    f = np.float32
    verts = [np.asarray(v, f) for v in (vertices_0, vertices_1, vertices_2)]
    sphs = [np.asarray(s, f) for s in (sph_0, sph_1, sph_2)]
    conn = np.asarray(connectivity)
    W2 = _assemble_W2([np.asarray(w, f) for w in (w_nl_0, w_nl_1, w_nl_2)])
    W3 = _assemble_W3([np.asarray(w, f) for w in (w_rel_0, w_rel_1, w_rel_2)])
    w2p = _pack_w2_tight(W2)
    w3p = _pack_chunked(W3, NCH3)

    in_maps = []
    for b in range(NB):
        vcat = np.concatenate([v[b].reshape(128, -1) for v in verts], axis=1)
        vcat_t = np.concatenate(
            [v[b].reshape(128, CH, LDIM[l]).transpose(0, 2, 1).reshape(128, -1)
             for l, v in enumerate(verts)], axis=1)
        cv = np.concatenate([conn[b].astype(f).T, vcat, vcat_t], axis=1)
        sph_cat = np.concatenate([s[b][:, :, 0, :] for s in sphs], axis=-1)
        sph_mj = sph_cat.transpose(0, 2, 1).reshape(128, 9 * 128)  # [i,(m,j)]
        in_maps.append(dict(
            cv=np.ascontiguousarray(cv.astype(ml_dtypes.bfloat16)),
            sph=np.ascontiguousarray(sph_mj.astype(ml_dtypes.bfloat16)),
            w2=w2p, w3=w3p))

    res = run_bass_kernel_spmd(_get_nc(), in_maps, list(range(NB)))
    LAST_RESULT["res"] = res
    Z = np.stack([res.results[b]["zout"] for b in range(NB)])   # [8, 128, 144]

    # host epilogue: unpack e=(l,c,k) cols, global per-l normalization
    out = np.zeros((NB, 128, 1, 16, 9), dtype=f)
    koff = [0, 1, 4]
    for l in range(3):
        cols = FOFF[l] + (np.arange(16)[:, None] * LDIM[l]
                          + np.arange(LDIM[l])[None, :])
        blk = Z[:, :, cols]                                     # [8,128,16,ld]
        nf = np.sum(blk.astype(np.float64) ** 2)
        out[:, :, 0, :, koff[l]:koff[l] + LDIM[l]] = blk / np.sqrt(nf / 16.0)
    return out
